# revision 1
# baseline (speedup 1.0000x reference)
# CrossAttention (B=2, S=2048, D=1024, H=16, dh=64) on 8 trn2 NeuronCores.
#
# Sharding: 32 (batch, head) units, 4 consecutive units per core (cores 0-3
# work on batch 0, cores 4-7 on batch 1). Each core receives its batch's
# hidden states pre-permuted to the on-chip [128, D/128, S] transposed
# layout, per-head slices of Wq/Wk/Wv/Wo (also pre-permuted); it returns a
# partial output y [2048, 1024] (its heads' contribution to the output
# projection). The host sums the four partials per batch and adds bo
# (tensor-parallel unshard of the output projection).
#
# Device algorithm (per core, 4 units = 2 pairs of heads), all matmuls
# fp32r (full PE rate at N>=256, fp32 accumulation). Triangular software
# pipeline over 512-wide s-block "rounds"; for round r:
#   - DMA hidden^T slice; project V (natural [s, feat] layout, stored as V'
#     with fused ones columns), K^T and Q^T (pair-packed [128, s]).
#   - run every attention cell (qb, kc) with max(qb, kc) == r; a cell is:
#     per pair, 4 key chunks of: S^T = K^T-chunk^T Q^T (two heads
#     row-packed, K=64 each), P^T = exp(S^T/8) on ACT (one [128,1024] op
#     covers both heads), O' += V'^T P^T (M=65: psum rows 0-63 unnormalized
#     out^T, row 64 = softmax denominator via the ones column); O' is
#     accumulated across rounds in SBUF.
#   - a cell per round is deferred and interleaved with the next round's
#     projections so the ACT engine stays fed.
# Finalize per q-block (interleaved with the last round's cells):
# normalize rows 0-63 by 1/(row 64) (DVE reciprocal -> K=1 ones-matmul
# partition broadcast -> DVE multiply), then output projection
# y += O_u^T Wo_u (K=64 per unit) and DMA out.
#
# PSUM plan (8 banks): S^T tiles 2x[128,1024], PV accumulators 2x[128,512],
# projection/broadcast/output transients 2x[128,512] - dedicated pools so
# the streams don't steal each other's slots.
import os
import sys

import numpy as np

try:
    import concourse.bass as bass
except ImportError:  # harness runs from a fresh dir; repo is on the default path
    sys.path.insert(0, "/opt/trn_rl_repo")
    import concourse.bass as bass

import concourse.bacc as bacc
import concourse.mybir as mybir
import concourse.tile as tile
from concourse.bass import ts, ds
from contextlib import ExitStack

B, S, D = 2, 2048, 1024
HEADS, DIM_HEAD = 16, 64
SCALE = DIM_HEAD**-0.5
N_CORES = 8
UNITS = 4  # (b, h) units per core
PAIRS = 2  # head pairs per core
P = 128
SB = S // 512  # 4 s-blocks of 512
QB = S // 512  # 4 q-blocks of 512
DC = D // P  # 8 contraction chunks for projections
KI = S // P  # 16 key chunks of 128
F32 = mybir.dt.float32
F32R = mybir.dt.float32r


def build_nc():
    nc = bacc.Bacc("TRN2", target_bir_lowering=False, debug=False)

    hiddent = nc.dram_tensor("hiddent", [P, DC, S], F32R, kind="ExternalInput").ap()
    # weights arrive pre-permuted to the on-chip layout (see shard_inputs),
    # declared fp32r end-to-end so the PE takes the full-rate matmul path
    wqt = nc.dram_tensor("wqt", [P, DC, 256], F32R, kind="ExternalInput").ap()
    wkt = nc.dram_tensor("wkt", [P, DC, 256], F32R, kind="ExternalInput").ap()
    wvt = nc.dram_tensor("wvt", [P, DC, 256], F32R, kind="ExternalInput").ap()
    wot = nc.dram_tensor("wot", [64, UNITS, D], F32R, kind="ExternalInput").ap()
    y = nc.dram_tensor("y", [S, D], F32, kind="ExternalOutput").ap()

    with tile.TileContext(nc) as tc, ExitStack() as ctx:
        persist = ctx.enter_context(tc.tile_pool(name="persist", bufs=1))
        pt_pool = ctx.enter_context(
            tc.tile_pool(name="pt", bufs=int(os.environ.get("K_PT", "6")))
        )
        otsb_pool = ctx.enter_context(
            tc.tile_pool(name="otsb", bufs=int(os.environ.get("K_OTSB", "4")))
        )
        rc_pool = ctx.enter_context(tc.tile_pool(name="rc", bufs=2))
        y_pool = ctx.enter_context(tc.tile_pool(name="ysb", bufs=2))
        # PSUM: "st" [128,1024] x2 = 4 banks (hidden transposes + S^T tiles);
        # "ot" [128,512] x4 = 4 banks (projection accums, PV accums,
        # broadcast and output-projection accums).
        st_ps = ctx.enter_context(tc.tile_pool(name="stps", bufs=int(os.environ.get("K_ST", "2")), space="PSUM"))
        # PSUM split: S^T tiles 2x[128,1024] (4 banks), PV accumulators
        # 2x[128,512] (2 banks), projection/broadcast/output transients
        # 2x[128,512] (2 banks). Dedicated pools keep cells and transients
        # from stealing each other's slots.
        ot_ps = ctx.enter_context(
            tc.tile_pool(name="otps", bufs=int(os.environ.get("K_OT", "2")), space="PSUM")
        )
        cell_ps = ctx.enter_context(
            tc.tile_pool(name="cellps", bufs=int(os.environ.get("K_CELL", "2")), space="PSUM")
        )

        # ---- persistent SBUF tensors ----
        KT = persist.tile([P, PAIRS, S], F32R)  # K^T pair-packed
        QT = persist.tile([P, PAIRS, S], F32R)  # Q^T pair-packed
        # V' per (k-chunk, pair): [V_unitA(64) | 1 | V_unitB(64) | 1];
        # each unit's PV is M=65 at base 0: out rows 0-63, sum at row 64
        Vp = persist.tile([P, KI, PAIRS, 130], F32R)
        wq_sb = persist.tile([P, DC, 256], F32R)
        wk_sb = persist.tile([P, DC, 256], F32R)
        wv_a = persist.tile([P, 4, 256], F32R)
        wv_b = persist.tile([P, 4, 256], F32R)
        wo_sb = persist.tile([64, UNITS, D], F32R)  # per-unit Wo rows (K=64)
        ones_sb = persist.tile([P, P], F32R)  # all-ones; row 64 = K=1 lhsT
        # O' accumulator: rows 0-63 unnormalized out^T, row 64 = denominator
        acc = persist.tile([65, QB, PAIRS, 2, 512], F32)
        # hidden^T lives only within its round
        ht_pool = ctx.enter_context(tc.tile_pool(name="htp", bufs=int(os.environ.get("K_HT", "2"))))

        # memset can't write fp32r; stage ones in f32 and round via copies
        ones_f32 = persist.tile([P, P], F32)
        nc.vector.memset(ones_f32, 1.0)
        # identity + weights go on the SWDGE queue so they don't
        # head-block the hidden-tile loads on the HWDGE queue
        # ordered by first use: V projection runs first in each round,
        # Wo isn't needed until the first finalize
        nc.gpsimd.dma_start(wv_a, wvt[:, 0:4, :])
        nc.gpsimd.dma_start(wv_b, wvt[:, 4:8, :])
        nc.gpsimd.dma_start(wk_sb, wkt)
        nc.gpsimd.dma_start(wq_sb, wqt)
        nc.gpsimd.dma_start(wo_sb, wot)
        nc.vector.tensor_copy(ones_sb, ones_f32)
        for col in (64, 129):
            nc.vector.tensor_copy(
                Vp[:, :, :, col : col + 1],
                ones_f32[:, 0:32].rearrange("p (a b c) -> p a b c", a=KI, b=PAIRS),
            )

        def attend_cell(qb, kc, pairs=None):
            """Attention for q-block qb against key chunks 4*kc..4*kc+3."""
            for p in pairs if pairs is not None else range(PAIRS):
                otA = cell_ps.tile([P, 512], F32, tag="ot")
                otB = cell_ps.tile([P, 512], F32, tag="ot")
                for k4 in range(4):
                    ki = kc * 4 + k4
                    stt = st_ps.tile([P, 1024], F32, tag="st")
                    nc.tensor.matmul(
                        stt[:, 0:512],
                        KT[0:64, p, ts(ki, 128)],
                        QT[0:64, p, ts(qb, 512)],
                        start=True,
                        stop=True,
                    )
                    nc.tensor.matmul(
                        stt[:, 512:1024],
                        KT[64:128, p, ts(ki, 128)],
                        QT[64:128, p, ts(qb, 512)],
                        start=True,
                        stop=True,
                    )
                    pt = pt_pool.tile([P, 1024], F32R)
                    nc.scalar.activation(
                        pt, stt, mybir.ActivationFunctionType.Exp, scale=SCALE
                    )
                    nc.tensor.matmul(
                        otA[0:65, :],
                        Vp[:, ki, p, 0:65],
                        pt[:, 0:512],
                        start=(k4 == 0),
                        stop=(k4 == 3),
                    )
                    nc.tensor.matmul(
                        otB[0:65, :],
                        Vp[:, ki, p, 65:130],
                        pt[:, 512:1024],
                        start=(k4 == 0),
                        stop=(k4 == 3),
                    )
                for u, ot in ((0, otA), (1, otB)):
                    sl = acc[:, qb, p, u, :]
                    if kc == 0:
                        nc.vector.tensor_copy(sl, ot[0:65, :])
                    else:
                        nc.vector.tensor_add(sl, sl, ot[0:65, :])

        def finalize(qb):
            """Normalize q-block qb and run its output projection."""
            ot_units = []
            for p in range(PAIRS):
                # one batched reciprocal covers both units of the pair
                rc = rc_pool.tile([65, 1024], F32R)
                with nc.allow_low_precision(
                    reason="fp32r rounding of softmax scale is plenty"
                ):
                    nc.vector.reciprocal(
                        rc[64:65, :],
                        acc[64:65, qb, p, :, :].rearrange("p a f -> p (a f)"),
                    )
                for u in range(2):
                    av = acc[:, qb, p, u, :]
                    rcs = rc[:, u * 512 : (u + 1) * 512]
                    bcp = (st_ps.tile([P, 1024], F32, tag="st", name="trans")[:, 0:512]
                           if os.environ.get("K_TRANS") == "st"
                           else ot_ps.tile([P, 512], F32, tag="ot"))
                    nc.tensor.matmul(
                        bcp, ones_sb[64:65, :], rcs[64:65, :], start=True, stop=True
                    )
                    otu = otsb_pool.tile([64, 512], F32R)
                    nc.vector.tensor_mul(otu, av[0:64, :], bcp[0:64, :])
                    ot_units.append(otu)
            for qt_i in range(4):
                for oh in range(2):
                    yps = (st_ps.tile([P, 1024], F32, tag="st", name="trans")[:, 0:512]
                           if os.environ.get("K_TRANS") == "st"
                           else ot_ps.tile([P, 512], F32, tag="ot"))
                    for u in range(UNITS):
                        nc.tensor.matmul(
                            yps,
                            ot_units[u][:, ts(qt_i, 128)],
                            wo_sb[:, u, ds(oh * 512, 512)],
                            start=(u == 0),
                            stop=(u == UNITS - 1),
                        )
                    ysb = y_pool.tile([P, 512], F32)
                    nc.vector.tensor_copy(ysb, yps)
                    nc.sync.dma_start(
                        y[qb * 512 + qt_i * 128 : qb * 512 + (qt_i + 1) * 128,
                          ds(oh * 512, 512)],
                        ysb,
                    )

        # ---- triangular pipeline: per s-block round, transpose + project,
        # then run every attention cell that just became ready ----
        deferred = []
        for sb in range(SB):
            # two separate half-tiles so the first projection matmuls
            # (dc 0-3) can start as soon as the first half lands (tile
            # pool dependencies are tile-granular)
            hTa = ht_pool.tile([P, 4, 512], F32R, tag="hta")
            hTb = ht_pool.tile([P, 4, 512], F32R, tag="htb")
            nc.sync.dma_start(hTa, hiddent[:, 0:4, ts(sb, 512)])
            nc.sync.dma_start(hTb, hiddent[:, 4:8, ts(sb, 512)])

            def hts(dc):
                return (hTa if dc < 4 else hTb)[:, dc % 4, :]
            for st in range(4):
                s0 = sb * 512 + st * 128
                ki_idx = sb * 4 + st
                # V projection for this s-tile (natural layout, all 4 units)
                vps = (st_ps.tile([P, 1024], F32, tag="st", name="trans")[:, 0:512]
                       if os.environ.get("K_TRANS") == "st"
                       else ot_ps.tile([P, 512], F32, tag="ot"))
                for dc in range(DC):
                    nc.tensor.matmul(
                        vps[:, :256],
                        hts(dc)[:, ts(st, 128)],
                        (wv_a if dc < 4 else wv_b)[:, dc % 4, :],
                        start=(dc == 0),
                        stop=(dc == DC - 1),
                    )
                for p in range(PAIRS):
                    nc.vector.tensor_copy(
                        Vp[:, ki_idx, p, 0:64], vps[:, (2 * p) * 64 : (2 * p + 1) * 64]
                    )
                    nc.vector.tensor_copy(
                        Vp[:, ki_idx, p, 65:129],
                        vps[:, (2 * p + 1) * 64 : (2 * p + 2) * 64],
                    )
            # K^T / Q^T projections for this s-block (pair-packed),
            # interleaved with cells deferred from the previous round so the
            # ACT engine stays fed while the PE runs projections
            projs = [(w, o, p) for (w, o) in ((wk_sb, KT), (wq_sb, QT))
                     for p in range(PAIRS)]
            for i, (w_sb, out_t, p) in enumerate(projs):
                kps = (st_ps.tile([P, 1024], F32, tag="st", name="trans")[:, 0:512]
                       if os.environ.get("K_TRANS") == "st"
                       else ot_ps.tile([P, 512], F32, tag="ot"))
                for dc in range(DC):
                    nc.tensor.matmul(
                        kps,
                        w_sb[:, dc, ts(p, 128)],
                        hts(dc),
                        start=(dc == 0),
                        stop=(dc == DC - 1),
                    )
                nc.vector.tensor_copy(out_t[:, p, ts(sb, 512)], kps)
                if i < len(deferred):
                    attend_cell(*deferred[i])
            deferred = []

            # newly-ready cells: earlier q-blocks against this round's keys,
            # plus this q-block against all keys so far
            new_cells = [(qb, sb) for qb in range(sb)]
            new_cells += [(sb, kc) for kc in range(sb + 1)]
            if sb < SB - 1:
                # defer the last N_DEFER cells, split per pair, to interleave
                # with the next round's projections
                n_defer = min(int(os.environ.get("K_DEFER", "1")), len(new_cells))
                if n_defer:
                    for qb, kc in new_cells[-n_defer:]:
                        for p in range(PAIRS):
                            deferred.append((qb, kc, [p]))
                    deferred = deferred[:4]
                    leftover = [
                        (qb, kc, [p])
                        for (qb, kc) in new_cells[-n_defer:]
                        for p in range(PAIRS)
                    ][4:]
                    new_cells = new_cells[:-n_defer]
                else:
                    leftover = []
                for cell in new_cells:
                    attend_cell(*cell)
                for qb, kc, ps in leftover:
                    attend_cell(qb, kc, ps)
            else:
                # last round: run this q-block's own cells first so its
                # finalize unlocks early, then finalize each q-block one
                # cell after its final cell lands, so finalize PE work
                # fills ACT-wait gaps of the in-flight cell
                if os.environ.get("K_LASTFIRST") == "1":
                    new_cells = new_cells[sb:] + new_cells[:sb]
                done = []
                for i, (qb, kc) in enumerate(new_cells):
                    attend_cell(qb, kc)
                    if done:
                        finalize(done.pop())
                    if (qb, kc) == (qb, SB - 1) and kc == SB - 1:
                        done.append(qb)
                for qb in done:
                    finalize(qb)
    nc.compile()
    return nc


_NC = None


def get_nc():
    global _NC
    if _NC is None:
        _NC = build_nc()
    return _NC


def shard_inputs(hidden_states, Wq, Wk, Wv, Wo):
    """Per-core input maps. Core c: batch c//4, heads 4*(c%4) .. 4*(c%4)+3."""
    hidden_states = np.asarray(hidden_states, np.float32)
    Wq, Wk, Wv, Wo = (np.asarray(w, np.float32) for w in (Wq, Wk, Wv, Wo))
    in_maps = []
    for c in range(N_CORES):
        b = c // 4
        f0 = (c % 4) * 4 * DIM_HEAD  # first feature row/col of this core's heads
        rows = slice(f0, f0 + UNITS * DIM_HEAD)

        def proj_layout(w):
            # W[rows].T is [D, 256]; on-chip layout is [128, DC, 256]
            return np.ascontiguousarray(
                w[rows, :].T.reshape(DC, P, 256).transpose(1, 0, 2)
            )

        # Wo[:, rows].T is [256, D]; on-chip layout is [64, UNITS, D]
        wot = np.ascontiguousarray(
            Wo[:, rows].T.reshape(UNITS, 64, D).transpose(1, 0, 2)
        )
        in_maps.append(
            {
                "hiddent": np.ascontiguousarray(
                    hidden_states[b].T.reshape(DC, P, S).transpose(1, 0, 2)
                ),
                "wqt": proj_layout(Wq),
                "wkt": proj_layout(Wk),
                "wvt": proj_layout(Wv),
                "wot": wot,
            }
        )
    return in_maps


def unshard_outputs(results, bo):
    out = np.zeros((B, S, D), np.float32)
    for c, res in enumerate(results):
        out[c // 4] += res["y"]
    out += np.asarray(bo, np.float32)[None, None, :]
    return out


def kernel(hidden_states, Wq, Wk, Wv, Wo, bo, _trace=False):
    from concourse.bass_utils import run_bass_kernel_spmd

    nc = get_nc()
    in_maps = shard_inputs(hidden_states, Wq, Wk, Wv, Wo)
    res = run_bass_kernel_spmd(nc, in_maps, list(range(N_CORES)), trace=_trace)
    out = unshard_outputs(res.results, bo)
    if _trace:
        return out, res
    return out



# revision 7
# speedup vs baseline: 1.0256x; 1.0256x over previous
# CrossAttention (B=2, S=2048, D=1024, H=16, dh=64) on 8 trn2 NeuronCores.
#
# Sharding: 32 (batch, head) units, 4 consecutive units per core (cores 0-3
# work on batch 0, cores 4-7 on batch 1). Each core receives its batch's
# hidden states pre-permuted to the on-chip [128, D/128, S] transposed
# layout, per-head slices of Wq/Wk/Wv/Wo (also pre-permuted); it returns a
# partial output y [2048, 1024] (its heads' contribution to the output
# projection). The host sums the four partials per batch and adds bo
# (tensor-parallel unshard of the output projection).
#
# Device algorithm (per core, 4 units = 2 pairs of heads), all matmuls
# fp32r (full PE rate at N>=256, fp32 accumulation). Triangular software
# pipeline over 512-wide s-block "rounds"; for round r:
#   - DMA hidden^T slice; project V (natural [s, feat] layout, stored as V'
#     with fused ones columns), K^T and Q^T (pair-packed [128, s]).
#   - run every attention cell (qb, kc) with max(qb, kc) == r; a cell is:
#     per pair, 4 key chunks of: S^T = K^T-chunk^T Q^T (two heads
#     row-packed, K=64 each), P^T = exp(S^T/8) on ACT (one [128,1024] op
#     covers both heads), O' += V'^T P^T (M=65: psum rows 0-63 unnormalized
#     out^T, row 64 = softmax denominator via the ones column); O' is
#     accumulated across rounds in SBUF.
#   - a cell per round is deferred and interleaved with the next round's
#     projections so the ACT engine stays fed.
# Finalize per q-block (interleaved with the last round's cells):
# normalize rows 0-63 by 1/(row 64) (DVE reciprocal -> GPSIMD partition
# broadcast on the otherwise-idle Pool engine -> DVE multiply into a
# [128, 512] pair-packed tile), then output projection with K=128
# (both units of a pair contracted in one matmul) and DMA out.
#
# PSUM plan (8 banks): S^T tiles 2x[128,1024] (4 banks), PV accumulators
# 2x[128,512] (2 banks), projection/output transients 2x[128,512] (2
# banks) - dedicated pools so the streams don't steal each other's slots.
import os
import sys

import numpy as np

try:
    import concourse.bass as bass
except ImportError:  # harness runs from a fresh dir; repo is on the default path
    sys.path.insert(0, "/opt/trn_rl_repo")
    import concourse.bass as bass

import concourse.bacc as bacc
import concourse.mybir as mybir
import concourse.tile as tile
from concourse.bass import ts, ds
from contextlib import ExitStack

B, S, D = 2, 2048, 1024
HEADS, DIM_HEAD = 16, 64
SCALE = DIM_HEAD**-0.5
N_CORES = 8
UNITS = 4  # (b, h) units per core
PAIRS = 2  # head pairs per core
P = 128
SB = S // 512  # 4 s-blocks of 512
QB = S // 512  # 4 q-blocks of 512
DC = D // P  # 8 contraction chunks for projections
KI = S // P  # 16 key chunks of 128
F32 = mybir.dt.float32
F32R = mybir.dt.float32r


def build_nc():
    nc = bacc.Bacc("TRN2", target_bir_lowering=False, debug=False)

    hiddent = nc.dram_tensor("hiddent", [P, DC, S], F32R, kind="ExternalInput").ap()
    # weights arrive pre-permuted to the on-chip layout (see shard_inputs),
    # declared fp32r end-to-end so the PE takes the full-rate matmul path
    wqt = nc.dram_tensor("wqt", [P, DC, 256], F32R, kind="ExternalInput").ap()
    wkt = nc.dram_tensor("wkt", [P, DC, 256], F32R, kind="ExternalInput").ap()
    wvt = nc.dram_tensor("wvt", [P, DC, 256], F32R, kind="ExternalInput").ap()
    # Wo pair-packed: partitions 0:64 = even unit of the pair, 64:128 = odd
    # unit, so the output projection contracts K=128 per matmul
    wot = nc.dram_tensor("wot", [P, PAIRS, D], F32R, kind="ExternalInput").ap()
    y = nc.dram_tensor("y", [S, D], F32, kind="ExternalOutput").ap()

    with tile.TileContext(nc) as tc, ExitStack() as ctx:
        persist = ctx.enter_context(tc.tile_pool(name="persist", bufs=1))
        pt_pool = ctx.enter_context(
            tc.tile_pool(name="pt", bufs=int(os.environ.get("K_PT", "6")))
        )
        otsb_pool = ctx.enter_context(
            tc.tile_pool(name="otsb", bufs=int(os.environ.get("K_OTSB", "4")))
        )
        rc_pool = ctx.enter_context(tc.tile_pool(name="rc", bufs=2))
        bc_pool = ctx.enter_context(tc.tile_pool(name="bc", bufs=2))
        y_pool = ctx.enter_context(tc.tile_pool(name="ysb", bufs=2))
        st_ps = ctx.enter_context(
            tc.tile_pool(name="stps", bufs=int(os.environ.get("K_ST", "2")), space="PSUM")
        )
        ot_ps = ctx.enter_context(
            tc.tile_pool(name="otps", bufs=int(os.environ.get("K_OT", "2")), space="PSUM")
        )
        cell_ps = ctx.enter_context(
            tc.tile_pool(name="cellps", bufs=int(os.environ.get("K_CELL", "2")), space="PSUM")
        )

        # ---- persistent SBUF tensors ----
        KT = persist.tile([P, PAIRS, S], F32R)  # K^T pair-packed
        QT = persist.tile([P, PAIRS, S], F32R)  # Q^T pair-packed
        # V' per (k-chunk, pair): [V_unitA(64) | 1 | V_unitB(64) | 1];
        # each unit's PV is M=65 at base 0: out rows 0-63, sum at row 64
        Vp = persist.tile([P, KI, PAIRS, 130], F32R)
        wq_sb = persist.tile([P, DC, 256], F32R)
        wk_sb = persist.tile([P, DC, 256], F32R)
        wv_a = persist.tile([P, 4, 256], F32R)
        wv_b = persist.tile([P, 4, 256], F32R)
        wo_sb = persist.tile([P, PAIRS, D], F32R)  # pair-packed Wo rows (K=128)
        # O' accumulator: rows 0-63 unnormalized out^T, row 64 = denominator
        acc = persist.tile([65, QB, PAIRS, 2, 512], F32)
        # hidden^T lives only within its round
        ht_pool = ctx.enter_context(
            tc.tile_pool(name="htp", bufs=int(os.environ.get("K_HT", "2")))
        )

        # memset can't write fp32r; stage ones in f32 and round via copies
        ones_f32 = persist.tile([P, P], F32)
        nc.vector.memset(ones_f32, 1.0)
        # weights go on the SWDGE queue so they don't head-block the
        # hidden-tile loads on the HWDGE queue; ordered by first use: V
        # projection runs first in each round, Wo isn't needed until the
        # first finalize
        nc.gpsimd.dma_start(wv_a, wvt[:, 0:4, :])
        nc.gpsimd.dma_start(wv_b, wvt[:, 4:8, :])
        nc.gpsimd.dma_start(wk_sb, wkt)
        nc.gpsimd.dma_start(wq_sb, wqt)
        nc.gpsimd.dma_start(wo_sb, wot)
        for col in (64, 129):
            nc.vector.tensor_copy(
                Vp[:, :, :, col : col + 1],
                ones_f32[:, 0:32].rearrange("p (a b c) -> p a b c", a=KI, b=PAIRS),
            )

        def attend_cell(qb, kc, pairs=None):
            """Attention for q-block qb against key chunks 4*kc..4*kc+3."""
            for p in pairs if pairs is not None else range(PAIRS):
                otA = cell_ps.tile([P, 512], F32, tag="ot")
                otB = cell_ps.tile([P, 512], F32, tag="ot")
                for k4 in range(4):
                    ki = kc * 4 + k4
                    stt = st_ps.tile([P, 1024], F32, tag="st")
                    nc.tensor.matmul(
                        stt[:, 0:512],
                        KT[0:64, p, ts(ki, 128)],
                        QT[0:64, p, ts(qb, 512)],
                        start=True,
                        stop=True,
                    )
                    nc.tensor.matmul(
                        stt[:, 512:1024],
                        KT[64:128, p, ts(ki, 128)],
                        QT[64:128, p, ts(qb, 512)],
                        start=True,
                        stop=True,
                    )
                    pt = pt_pool.tile([P, 1024], F32R)
                    nc.scalar.activation(
                        pt, stt, mybir.ActivationFunctionType.Exp, scale=SCALE
                    )
                    nc.tensor.matmul(
                        otA[0:65, :],
                        Vp[:, ki, p, 0:65],
                        pt[:, 0:512],
                        start=(k4 == 0),
                        stop=(k4 == 3),
                    )
                    nc.tensor.matmul(
                        otB[0:65, :],
                        Vp[:, ki, p, 65:130],
                        pt[:, 512:1024],
                        start=(k4 == 0),
                        stop=(k4 == 3),
                    )
                for u, ot in ((0, otA), (1, otB)):
                    sl = acc[:, qb, p, u, :]
                    if kc == 0:
                        nc.vector.tensor_copy(sl, ot[0:65, :])
                    else:
                        nc.vector.tensor_add(sl, sl, ot[0:65, :])

        def finalize(qb):
            """Normalize q-block qb and run its output projection."""
            ot_pairs = []
            for p in range(PAIRS):
                # one batched reciprocal covers both units of the pair; the
                # result lands at partition 0 — the hardware partition
                # broadcast does not honor a partition-offset input
                rc = rc_pool.tile([1, 1024], F32R)
                with nc.allow_low_precision(
                    reason="fp32r rounding of softmax scale is plenty"
                ):
                    nc.vector.reciprocal(
                        rc,
                        acc[64:65, qb, p, :, :].rearrange("p a f -> p (a f)"),
                    )
                # broadcast 1/denom across partitions on the idle Pool engine
                bc = bc_pool.tile([64, 1024], F32R)
                nc.gpsimd.partition_broadcast(bc, rc)
                # pair-packed normalized O^T: partitions 0:64 = even unit,
                # 64:128 = odd unit, so the output projection gets K=128
                otp = otsb_pool.tile([P, 512], F32R)
                nc.vector.tensor_mul(otp[0:64, :], acc[0:64, qb, p, 0, :], bc[:, 0:512])
                nc.vector.tensor_mul(
                    otp[64:128, :], acc[0:64, qb, p, 1, :], bc[:, 512:1024]
                )
                ot_pairs.append(otp)
            for qt_i in range(4):
                for oh in range(2):
                    yps = ot_ps.tile([P, 512], F32, tag="ot")
                    for p in range(PAIRS):
                        nc.tensor.matmul(
                            yps,
                            ot_pairs[p][:, ts(qt_i, 128)],
                            wo_sb[:, p, ds(oh * 512, 512)],
                            start=(p == 0),
                            stop=(p == PAIRS - 1),
                        )
                    ysb = y_pool.tile([P, 512], F32)
                    nc.vector.tensor_copy(ysb, yps)
                    nc.sync.dma_start(
                        y[qb * 512 + qt_i * 128 : qb * 512 + (qt_i + 1) * 128,
                          ds(oh * 512, 512)],
                        ysb,
                    )

        # ---- triangular pipeline: per s-block round, transpose + project,
        # then run every attention cell that just became ready ----
        deferred = []
        for sb in range(SB):
            # four separate tiles so the first projection matmuls can start
            # as soon as the first chunk lands (tile pool dependencies are
            # tile-granular); round 0 splits 1/3/2/2 to cut the cold-start
            # latency of the very first matmul
            bounds = [0, 1, 4, 6, 8] if sb == 0 else [0, 2, 4, 6, 8]
            hT_tiles = []
            for i in range(4):
                lo, hi = bounds[i], bounds[i + 1]
                htt = ht_pool.tile([P, hi - lo, 512], F32R, tag=f"ht{i}", name=f"ht{i}")
                hT_tiles.append((htt, lo, hi))
            for t, lo, hi in hT_tiles:
                nc.sync.dma_start(t, hiddent[:, lo:hi, ts(sb, 512)])

            def hts(dc):
                for t, lo, hi in hT_tiles:
                    if lo <= dc < hi:
                        return t[:, dc - lo, :]
                raise AssertionError(dc)

            for st in range(4):
                s0 = sb * 512 + st * 128
                ki_idx = sb * 4 + st
                # V projection for this s-tile (natural layout, all 4 units)
                vps = ot_ps.tile([P, 512], F32, tag="ot")
                for dc in range(DC):
                    nc.tensor.matmul(
                        vps[:, :256],
                        hts(dc)[:, ts(st, 128)],
                        (wv_a if dc < 4 else wv_b)[:, dc % 4, :],
                        start=(dc == 0),
                        stop=(dc == DC - 1),
                    )
                for p in range(PAIRS):
                    nc.vector.tensor_copy(
                        Vp[:, ki_idx, p, 0:64], vps[:, (2 * p) * 64 : (2 * p + 1) * 64]
                    )
                    nc.vector.tensor_copy(
                        Vp[:, ki_idx, p, 65:129],
                        vps[:, (2 * p + 1) * 64 : (2 * p + 2) * 64],
                    )
            # K^T / Q^T projections for this s-block (pair-packed),
            # interleaved with cells deferred from the previous round so the
            # ACT engine stays fed while the PE runs projections
            projs = [(w, o, p) for (w, o) in ((wk_sb, KT), (wq_sb, QT))
                     for p in range(PAIRS)]
            for i, (w_sb, out_t, p) in enumerate(projs):
                kps = ot_ps.tile([P, 512], F32, tag="ot")
                for dc in range(DC):
                    nc.tensor.matmul(
                        kps,
                        w_sb[:, dc, ts(p, 128)],
                        hts(dc),
                        start=(dc == 0),
                        stop=(dc == DC - 1),
                    )
                nc.vector.tensor_copy(out_t[:, p, ts(sb, 512)], kps)
                if i < len(deferred):
                    attend_cell(*deferred[i])
            deferred = []

            # newly-ready cells: earlier q-blocks against this round's keys,
            # plus this q-block against all keys so far
            new_cells = [(qb, sb) for qb in range(sb)]
            new_cells += [(sb, kc) for kc in range(sb + 1)]
            if sb < SB - 1:
                # defer the last N_DEFER cells, split per pair, to interleave
                # with the next round's projections
                n_defer = min(int(os.environ.get("K_DEFER", "1")), len(new_cells))
                if n_defer:
                    for qb, kc in new_cells[-n_defer:]:
                        for p in range(PAIRS):
                            deferred.append((qb, kc, [p]))
                    deferred = deferred[:4]
                    leftover = [
                        (qb, kc, [p])
                        for (qb, kc) in new_cells[-n_defer:]
                        for p in range(PAIRS)
                    ][4:]
                    new_cells = new_cells[:-n_defer]
                else:
                    leftover = []
                for cell in new_cells:
                    attend_cell(*cell)
                for qb, kc, ps in leftover:
                    attend_cell(qb, kc, ps)
            else:
                # last round: run this q-block's own cells first so its
                # finalize unlocks early, then finalize each q-block one
                # cell after its final cell lands, so finalize PE work
                # fills ACT-wait gaps of the in-flight cell
                if os.environ.get("K_LASTFIRST") == "1":
                    new_cells = new_cells[sb:] + new_cells[:sb]
                done = []
                for i, (qb, kc) in enumerate(new_cells):
                    attend_cell(qb, kc)
                    if done:
                        finalize(done.pop())
                    if kc == SB - 1:
                        done.append(qb)
                for qb in done:
                    finalize(qb)
    nc.compile()
    return nc


_NC = None


def get_nc():
    global _NC
    if _NC is None:
        _NC = build_nc()
    return _NC


def shard_inputs(hidden_states, Wq, Wk, Wv, Wo):
    """Per-core input maps. Core c: batch c//4, heads 4*(c%4) .. 4*(c%4)+3."""
    hidden_states = np.asarray(hidden_states, np.float32)
    Wq, Wk, Wv, Wo = (np.asarray(w, np.float32) for w in (Wq, Wk, Wv, Wo))
    in_maps = []
    for c in range(N_CORES):
        b = c // 4
        f0 = (c % 4) * 4 * DIM_HEAD  # first feature row/col of this core's heads
        rows = slice(f0, f0 + UNITS * DIM_HEAD)

        def proj_layout(w):
            # W[rows].T is [D, 256]; on-chip layout is [128, DC, 256]
            return np.ascontiguousarray(
                w[rows, :].T.reshape(DC, P, 256).transpose(1, 0, 2)
            )

        # Wo[:, rows].T is [256, D]; pair-packed on-chip layout [128, PAIRS, D]
        wot = np.ascontiguousarray(
            Wo[:, rows].T.reshape(PAIRS, P, D).transpose(1, 0, 2)
        )
        in_maps.append(
            {
                "hiddent": np.ascontiguousarray(
                    hidden_states[b].T.reshape(DC, P, S).transpose(1, 0, 2)
                ),
                "wqt": proj_layout(Wq),
                "wkt": proj_layout(Wk),
                "wvt": proj_layout(Wv),
                "wot": wot,
            }
        )
    return in_maps


def unshard_outputs(results, bo):
    out = np.zeros((B, S, D), np.float32)
    for c, res in enumerate(results):
        out[c // 4] += res["y"]
    out += np.asarray(bo, np.float32)[None, None, :]
    return out


def kernel(hidden_states, Wq, Wk, Wv, Wo, bo, _trace=False):
    from concourse.bass_utils import run_bass_kernel_spmd

    nc = get_nc()
    in_maps = shard_inputs(hidden_states, Wq, Wk, Wv, Wo)
    res = run_bass_kernel_spmd(nc, in_maps, list(range(N_CORES)), trace=_trace)
    out = unshard_outputs(res.results, bo)
    if _trace:
        return out, res
    return out


# revision 8
# speedup vs baseline: 1.0667x; 1.0401x over previous
# CrossAttention (B=2, S=2048, D=1024, H=16, dh=64) on 8 trn2 NeuronCores.
#
# Sharding: 32 (batch, head) units, 4 consecutive units per core (cores 0-3
# work on batch 0, cores 4-7 on batch 1). Each core receives its batch's
# hidden states pre-permuted to the on-chip [128, D/128, S] transposed
# layout, per-head slices of Wq/Wk/Wv/Wo (also pre-permuted); it returns a
# partial output y [2048, 1024] (its heads' contribution to the output
# projection). The host sums the four partials per batch and adds bo
# (tensor-parallel unshard of the output projection).
#
# Device algorithm (per core, 4 units = 2 pairs of heads), all matmuls
# fp32r (full PE rate at N>=256, fp32 accumulation).
#
# Schedule: the PE executes in order, so ACT-bound attention cells (the
# exp chain of a cell-pair takes ~4.2us vs ~3.4us of PE work) are woven
# between PE-only projection chains at pair granularity: every round's
# cells are deferred one round and interleaved with the next round's
# V/K/Q projection chains, keeping the PE fed while ACT catches up. The
# last round's cells interleave with the finalizes (normalize + output
# projection), whose PE work plays the same filler role.
#
# A cell (qb, kc), per pair: 4 key chunks of S^T = K^T-chunk^T Q^T (two
# heads row-packed, K=64 each), P^T = exp(S^T/8) on ACT (one [128,1024]
# op covers both heads), O' += V'^T P^T (M=65: psum rows 0-63
# unnormalized out^T, row 64 = softmax denominator via a fused ones
# column in V'); O' accumulates across rounds in SBUF.
#
# Finalize per q-block: DVE reciprocal of the denominators (written to
# partition 0), GPSIMD partition-broadcast on the otherwise-idle Pool
# engine, DVE multiply into a [128, 512] pair-packed tile (partitions
# 0:64 even unit, 64:128 odd unit), then the output projection contracts
# K=128 per matmul; PSUM->SBUF output copies alternate DVE/ACT so neither
# engine paces the tail, and the final store is split in half so the
# last DMA transfer off the critical path is short.
#
# PSUM plan (8 banks): S^T tiles 2x[128,1024] (4 banks), PV accumulators
# 2x[128,512] (2 banks), projection/output transients 2x[128,512] (2
# banks) - dedicated pools so the streams don't steal each other's slots.
import os
import sys

import numpy as np

try:
    import concourse.bass as bass
except ImportError:  # harness runs from a fresh dir; repo is on the default path
    sys.path.insert(0, "/opt/trn_rl_repo")
    import concourse.bass as bass

import concourse.bacc as bacc
import concourse.mybir as mybir
import concourse.tile as tile
from concourse.bass import ts, ds
from contextlib import ExitStack

B, S, D = 2, 2048, 1024
HEADS, DIM_HEAD = 16, 64
SCALE = DIM_HEAD**-0.5
N_CORES = 8
UNITS = 4  # (b, h) units per core
PAIRS = 2  # head pairs per core
P = 128
SB = S // 512  # 4 s-blocks of 512
QB = S // 512  # 4 q-blocks of 512
DC = D // P  # 8 contraction chunks for projections
KI = S // P  # 16 key chunks of 128
F32 = mybir.dt.float32
F32R = mybir.dt.float32r


def build_nc():
    nc = bacc.Bacc("TRN2", target_bir_lowering=False, debug=False)

    hiddent = nc.dram_tensor("hiddent", [P, DC, S], F32R, kind="ExternalInput").ap()
    # weights arrive pre-permuted to the on-chip layout (see shard_inputs),
    # declared fp32r end-to-end so the PE takes the full-rate matmul path
    wqt = nc.dram_tensor("wqt", [P, DC, 256], F32R, kind="ExternalInput").ap()
    wkt = nc.dram_tensor("wkt", [P, DC, 256], F32R, kind="ExternalInput").ap()
    wvt = nc.dram_tensor("wvt", [P, DC, 256], F32R, kind="ExternalInput").ap()
    # Wo pair-packed: partitions 0:64 = even unit of the pair, 64:128 = odd
    # unit, so the output projection contracts K=128 per matmul
    wot = nc.dram_tensor("wot", [P, PAIRS, D], F32R, kind="ExternalInput").ap()
    y = nc.dram_tensor("y", [S, D], F32, kind="ExternalOutput").ap()

    with tile.TileContext(nc) as tc, ExitStack() as ctx:
        persist = ctx.enter_context(tc.tile_pool(name="persist", bufs=1))
        pt_pool = ctx.enter_context(
            tc.tile_pool(name="pt", bufs=int(os.environ.get("K_PT", "6")))
        )
        otsb_pool = ctx.enter_context(
            tc.tile_pool(name="otsb", bufs=int(os.environ.get("K_OTSB", "4")))
        )
        rc_pool = ctx.enter_context(tc.tile_pool(name="rc", bufs=2))
        bc_pool = ctx.enter_context(tc.tile_pool(name="bc", bufs=2))
        y_pool = ctx.enter_context(
            tc.tile_pool(name="ysb", bufs=int(os.environ.get("K_Y", "3")))
        )
        st_ps = ctx.enter_context(
            tc.tile_pool(name="stps", bufs=int(os.environ.get("K_ST", "2")), space="PSUM")
        )
        ot_ps = ctx.enter_context(
            tc.tile_pool(name="otps", bufs=int(os.environ.get("K_OT", "2")), space="PSUM")
        )
        cell_ps = ctx.enter_context(
            tc.tile_pool(name="cellps", bufs=int(os.environ.get("K_CELL", "2")), space="PSUM")
        )

        # ---- persistent SBUF tensors ----
        KT = persist.tile([P, PAIRS, S], F32R)  # K^T pair-packed
        QT = persist.tile([P, PAIRS, S], F32R)  # Q^T pair-packed
        # V' per (k-chunk, pair): [V_unitA(64) | 1 | V_unitB(64) | 1];
        # each unit's PV is M=65 at base 0: out rows 0-63, sum at row 64
        Vp = persist.tile([P, KI, PAIRS, 130], F32R)
        wq_sb = persist.tile([P, DC, 256], F32R)
        wk_sb = persist.tile([P, DC, 256], F32R)
        wv_a = persist.tile([P, 4, 256], F32R)
        wv_b = persist.tile([P, 4, 256], F32R)
        wo_sb = persist.tile([P, PAIRS, D], F32R)  # pair-packed Wo rows (K=128)
        # O' accumulator: rows 0-63 unnormalized out^T, row 64 = denominator
        acc = persist.tile([65, QB, PAIRS, 2, 512], F32)
        # hidden^T lives only within its round
        ht_pool = ctx.enter_context(
            tc.tile_pool(name="htp", bufs=int(os.environ.get("K_HT", "2")))
        )

        # memset can't write fp32r; stage ones in f32 and round via copies
        ones_f32 = persist.tile([P, P], F32)
        nc.vector.memset(ones_f32, 1.0)
        # weights go on the SWDGE queue so they don't head-block the
        # hidden-tile loads on the HWDGE queue; ordered by first use: V
        # projection runs first in round 0, Wo isn't needed until the
        # first finalize
        nc.gpsimd.dma_start(wv_a, wvt[:, 0:4, :])
        nc.gpsimd.dma_start(wv_b, wvt[:, 4:8, :])
        nc.gpsimd.dma_start(wk_sb, wkt)
        nc.gpsimd.dma_start(wq_sb, wqt)
        nc.gpsimd.dma_start(wo_sb, wot)
        for col in (64, 129):
            nc.vector.tensor_copy(
                Vp[:, :, :, col : col + 1],
                ones_f32[:, 0:32].rearrange("p (a b c) -> p a b c", a=KI, b=PAIRS),
            )

        def cell_pair(qb, kc, p):
            """One head-pair of the attention cell (qb, kc)."""
            otA = cell_ps.tile([P, 512], F32, tag="ot", name="otA")
            otB = cell_ps.tile([P, 512], F32, tag="ot", name="otB")
            for k4 in range(4):
                ki = kc * 4 + k4
                stt = st_ps.tile([P, 1024], F32, tag="st")
                nc.tensor.matmul(
                    stt[:, 0:512],
                    KT[0:64, p, ts(ki, 128)],
                    QT[0:64, p, ts(qb, 512)],
                    start=True,
                    stop=True,
                )
                nc.tensor.matmul(
                    stt[:, 512:1024],
                    KT[64:128, p, ts(ki, 128)],
                    QT[64:128, p, ts(qb, 512)],
                    start=True,
                    stop=True,
                )
                pt = pt_pool.tile([P, 1024], F32R)
                nc.scalar.activation(
                    pt, stt, mybir.ActivationFunctionType.Exp, scale=SCALE
                )
                nc.tensor.matmul(
                    otA[0:65, :],
                    Vp[:, ki, p, 0:65],
                    pt[:, 0:512],
                    start=(k4 == 0),
                    stop=(k4 == 3),
                )
                nc.tensor.matmul(
                    otB[0:65, :],
                    Vp[:, ki, p, 65:130],
                    pt[:, 512:1024],
                    start=(k4 == 0),
                    stop=(k4 == 3),
                )
            for u, ot in ((0, otA), (1, otB)):
                sl = acc[:, qb, p, u, :]
                if kc == 0:
                    nc.vector.tensor_copy(sl, ot[0:65, :])
                else:
                    nc.vector.tensor_add(sl, sl, ot[0:65, :])

        ot_pairs = {}  # (qb, p) -> normalized pair-packed O^T tile

        def normalize(qb, p):
            """Reciprocal + broadcast + scale for one (q-block, pair).

            DVE/Pool only — contributes no PE work, so it can be emitted
            immediately after the pair's last cell without stalling the PE.
            """
            # one batched reciprocal covers both units of the pair; the
            # result lands at partition 0 — the hardware partition
            # broadcast does not honor a partition-offset input
            rc = rc_pool.tile([1, 1024], F32R)
            with nc.allow_low_precision(
                reason="fp32r rounding of softmax scale is plenty"
            ):
                nc.vector.reciprocal(
                    rc,
                    acc[64:65, qb, p, :, :].rearrange("p a f -> p (a f)"),
                )
            # broadcast 1/denom across partitions on the idle Pool engine
            bc = bc_pool.tile([64, 1024], F32R)
            nc.gpsimd.partition_broadcast(bc, rc)
            # pair-packed normalized O^T: partitions 0:64 = even unit,
            # 64:128 = odd unit, so the output projection gets K=128
            otp = otsb_pool.tile([P, 512], F32R)
            nc.vector.tensor_mul(otp[0:64, :], acc[0:64, qb, p, 0, :], bc[:, 0:512])
            nc.vector.tensor_mul(otp[64:128, :], acc[0:64, qb, p, 1, :], bc[:, 512:1024])
            ot_pairs[(qb, p)] = otp

        def outproj(qb, groups, split_last=False):
            """Output projection for q-block qb over (qt_i, oh) groups."""
            for gi, (qt_i, oh) in enumerate(groups):
                yps = ot_ps.tile([P, 512], F32, tag="ot")
                for p in range(PAIRS):
                    nc.tensor.matmul(
                        yps,
                        ot_pairs[(qb, p)][:, ts(qt_i, 128)],
                        wo_sb[:, p, ds(oh * 512, 512)],
                        start=(p == 0),
                        stop=(p == PAIRS - 1),
                    )
                ysb = y_pool.tile([P, 512], F32)
                # alternate the PSUM->SBUF copy between DVE and ACT (Copy
                # shares Exp's activation table - no table reload)
                if (qt_i * 2 + oh) % 2 == 0:
                    nc.vector.tensor_copy(ysb, yps)
                else:
                    nc.scalar.activation(
                        ysb, yps, mybir.ActivationFunctionType.Copy, scale=1.0
                    )
                rows = y[qb * 512 + qt_i * 128 : qb * 512 + (qt_i + 1) * 128, :]
                if split_last and gi == len(groups) - 1:
                    nc.sync.dma_start(rows[:, ds(oh * 512, 256)], ysb[:, 0:256])
                    nc.sync.dma_start(rows[:, ds(oh * 512 + 256, 256)], ysb[:, 256:512])
                else:
                    nc.sync.dma_start(rows[:, ds(oh * 512, 512)], ysb)

        # ---- pipeline: per s-block round, DMA + project, with the
        # PREVIOUS round's cells woven between projection chains at
        # head-pair granularity so the in-order PE always has
        # ACT-independent work while the exp chain catches up ----
        def interleave(chains, pairs):
            """Emit chain thunks with pair thunks distributed evenly."""
            n_c, n_p = len(chains), len(pairs)
            if n_c == 0:
                for t in pairs:
                    t()
                return
            # pairs after chain i: spread n_p slots over n_c positions
            emitted = 0
            for i, ch in enumerate(chains):
                ch()
                want = (i + 1) * n_p // n_c
                while emitted < want:
                    pairs[emitted]()
                    emitted += 1

        pending = []  # cell-pair thunks deferred from the previous round
        for sb in range(SB):
            bounds = [0, 1, 4, 6, 8] if sb == 0 else [0, 2, 4, 6, 8]
            hT_tiles = []
            for i in range(4):
                lo, hi = bounds[i], bounds[i + 1]
                htt = ht_pool.tile([P, hi - lo, 512], F32R, tag=f"ht{i}", name=f"ht{i}")
                hT_tiles.append((htt, lo, hi))
            for t, lo, hi in hT_tiles:
                nc.sync.dma_start(t, hiddent[:, lo:hi, ts(sb, 512)])

            def hts(dc, hT_tiles=hT_tiles):
                for t, lo, hi in hT_tiles:
                    if lo <= dc < hi:
                        return t[:, dc - lo, :]
                raise AssertionError(dc)

            def v_chain(st, sb=sb, hts=hts):
                ki_idx = sb * 4 + st
                vps = ot_ps.tile([P, 512], F32, tag="ot", name="vps")
                for dc in range(DC):
                    nc.tensor.matmul(
                        vps[:, :256],
                        hts(dc)[:, ts(st, 128)],
                        (wv_a if dc < 4 else wv_b)[:, dc % 4, :],
                        start=(dc == 0),
                        stop=(dc == DC - 1),
                    )
                for p in range(PAIRS):
                    nc.vector.tensor_copy(
                        Vp[:, ki_idx, p, 0:64], vps[:, (2 * p) * 64 : (2 * p + 1) * 64]
                    )
                    nc.vector.tensor_copy(
                        Vp[:, ki_idx, p, 65:129],
                        vps[:, (2 * p + 1) * 64 : (2 * p + 2) * 64],
                    )

            def kq_chain(w_sb, out_t, p, sb=sb, hts=hts):
                kps = ot_ps.tile([P, 512], F32, tag="ot", name="kps")
                for dc in range(DC):
                    nc.tensor.matmul(
                        kps,
                        w_sb[:, dc, ts(p, 128)],
                        hts(dc),
                        start=(dc == 0),
                        stop=(dc == DC - 1),
                    )
                nc.vector.tensor_copy(out_t[:, p, ts(sb, 512)], kps)

            chains = [lambda st=st: v_chain(st) for st in range(4)]
            chains += [
                lambda w=w, o=o, p=p: kq_chain(w, o, p)
                for (w, o) in ((wk_sb, KT), (wq_sb, QT))
                for p in range(PAIRS)
            ]
            interleave(chains, pending)

            new_cells = [(qb, sb) for qb in range(sb)]
            new_cells += [(sb, kc) for kc in range(sb + 1)]
            pending = [
                lambda qb=qb, kc=kc, p=p: cell_pair(qb, kc, p)
                for (qb, kc) in new_cells
                for p in range(PAIRS)
            ]

        # ---- tail: the last round's cells (q-block 3 first so its
        # finalize unlocks early), with normalizes emitted per-pair as
        # their data lands and output projections split in halves to
        # fill the remaining cells' ACT-wait gaps ----
        tail_cells = [(3, 0), (3, 1), (3, 2), (3, 3), (0, 3), (1, 3), (2, 3)]
        g_lo = [(0, 0), (0, 1), (1, 0), (1, 1)]
        g_hi = [(2, 0), (2, 1), (3, 0), (3, 1)]
        post = {  # emission index (qb, kc, p) -> actions after that pair
            (3, 3, 0): [lambda: normalize(3, 0)],
            (3, 3, 1): [lambda: normalize(3, 1)],
            (0, 3, 0): [lambda: normalize(0, 0), lambda: outproj(3, g_lo)],
            (0, 3, 1): [lambda: normalize(0, 1), lambda: outproj(3, g_hi)],
            (1, 3, 0): [lambda: normalize(1, 0), lambda: outproj(0, g_lo)],
            (1, 3, 1): [lambda: normalize(1, 1), lambda: outproj(0, g_hi)],
            (2, 3, 0): [lambda: normalize(2, 0), lambda: outproj(1, g_lo)],
            (2, 3, 1): [lambda: normalize(2, 1), lambda: outproj(1, g_hi)],
        }
        for qb, kc in tail_cells:
            for p in range(PAIRS):
                cell_pair(qb, kc, p)
                for act in post.get((qb, kc, p), []):
                    act()
        outproj(2, g_lo + g_hi, split_last=True)
    nc.compile()
    return nc


_NC = None


def get_nc():
    global _NC
    if _NC is None:
        _NC = build_nc()
    return _NC


def shard_inputs(hidden_states, Wq, Wk, Wv, Wo):
    """Per-core input maps. Core c: batch c//4, heads 4*(c%4) .. 4*(c%4)+3."""
    hidden_states = np.asarray(hidden_states, np.float32)
    Wq, Wk, Wv, Wo = (np.asarray(w, np.float32) for w in (Wq, Wk, Wv, Wo))
    in_maps = []
    for c in range(N_CORES):
        b = c // 4
        f0 = (c % 4) * 4 * DIM_HEAD  # first feature row/col of this core's heads
        rows = slice(f0, f0 + UNITS * DIM_HEAD)

        def proj_layout(w):
            # W[rows].T is [D, 256]; on-chip layout is [128, DC, 256]
            return np.ascontiguousarray(
                w[rows, :].T.reshape(DC, P, 256).transpose(1, 0, 2)
            )

        # Wo[:, rows].T is [256, D]; pair-packed on-chip layout [128, PAIRS, D]
        wot = np.ascontiguousarray(
            Wo[:, rows].T.reshape(PAIRS, P, D).transpose(1, 0, 2)
        )
        in_maps.append(
            {
                "hiddent": np.ascontiguousarray(
                    hidden_states[b].T.reshape(DC, P, S).transpose(1, 0, 2)
                ),
                "wqt": proj_layout(Wq),
                "wkt": proj_layout(Wk),
                "wvt": proj_layout(Wv),
                "wot": wot,
            }
        )
    return in_maps


def unshard_outputs(results, bo):
    out = np.zeros((B, S, D), np.float32)
    for c, res in enumerate(results):
        out[c // 4] += res["y"]
    out += np.asarray(bo, np.float32)[None, None, :]
    return out


def kernel(hidden_states, Wq, Wk, Wv, Wo, bo, _trace=False):
    from concourse.bass_utils import run_bass_kernel_spmd

    nc = get_nc()
    in_maps = shard_inputs(hidden_states, Wq, Wk, Wv, Wo)
    res = run_bass_kernel_spmd(nc, in_maps, list(range(N_CORES)), trace=_trace)
    out = unshard_outputs(res.results, bo)
    if _trace:
        return out, res
    return out


# revision 25
# speedup vs baseline: 1.1408x; 1.0695x over previous
# CrossAttention (B=2, S=2048, D=1024, H=16, dh=64) on 8 trn2 NeuronCores.
#
# Sharding: 32 (batch, head) units, 4 consecutive units per core (cores 0-3
# work on batch 0, cores 4-7 on batch 1). Each core receives its batch's
# hidden states pre-permuted to the on-chip [128, D/128, S] transposed
# layout, per-head slices of Wq/Wk/Wv/Wo (also pre-permuted); it returns a
# partial output y [2048, 1024] (its heads' contribution to the output
# projection). The host sums the four partials per batch and adds bo
# (tensor-parallel unshard of the output projection).
#
# Device algorithm (per core, 4 units = 2 pairs of heads), all matmuls
# fp32r (full PE rate at N>=256, fp32 accumulation).
#
# Schedule: the PE executes in order, so ACT-bound attention cells (the
# exp chain of a cell-pair takes ~4.2us vs ~3.4us of PE work) are woven
# between PE-only projection chains at pair granularity: every round's
# cells are deferred one round and interleaved with the next round's
# V/K/Q projection chains, keeping the PE fed while ACT catches up. The
# last round's cells interleave with the finalizes (normalize + output
# projection), whose PE work plays the same filler role.
#
# A cell (qb, kc), per pair: 4 key chunks of S^T = K^T-chunk^T Q^T (two
# heads row-packed, K=64 each), P^T = exp(S^T/8) on ACT (one [128,1024]
# op covers both heads), O' += V'^T P^T (M=65: psum rows 0-63
# unnormalized out^T, row 64 = softmax denominator via a fused ones
# column in V'); O' accumulates across rounds in SBUF.
#
# Finalize per q-block: DVE reciprocal of the denominators (written to
# partition 0), GPSIMD partition-broadcast on the otherwise-idle Pool
# engine, DVE multiply into a [128, 512] pair-packed tile (partitions
# 0:64 even unit, 64:128 odd unit), then the output projection contracts
# K=128 per matmul; PSUM->SBUF output copies alternate DVE/ACT so neither
# engine paces the tail, and the final store is split in half so the
# last DMA transfer off the critical path is short.
#
# PSUM plan (8 banks): S^T tiles 2x[128,1024] (4 banks), PV accumulators
# 2x[128,512] (2 banks), projection/output transients 2x[128,512] (2
# banks) - dedicated pools so the streams don't steal each other's slots.
import os
import sys

import numpy as np

try:
    import concourse.bass as bass
except ImportError:  # harness runs from a fresh dir; repo is on the default path
    sys.path.insert(0, "/opt/trn_rl_repo")
    import concourse.bass as bass

import concourse.bacc as bacc
import concourse.mybir as mybir
import concourse.tile as tile
from concourse.bass import ts, ds
from contextlib import ExitStack

B, S, D = 2, 2048, 1024
HEADS, DIM_HEAD = 16, 64
SCALE = DIM_HEAD**-0.5
N_CORES = 8
UNITS = 4  # (b, h) units per core
PAIRS = 2  # head pairs per core
P = 128
SB = S // 512  # 4 s-blocks of 512
QB = S // 512  # 4 q-blocks of 512
DC = D // P  # 8 contraction chunks for projections
KI = S // P  # 16 key chunks of 128
F32 = mybir.dt.float32
F32R = mybir.dt.float32r
BF16 = mybir.dt.bfloat16


def build_nc():
    nc = bacc.Bacc("TRN2", target_bir_lowering=False, debug=False)

    # hidden and the QKV projection weights ship as bf16: the projection
    # matmuls are pure bf16 x bf16 (full PE rate, fp32 accumulation) and
    # the input DMA bytes halve, which is what paces the cold start
    hiddent = nc.dram_tensor("hiddent", [P, DC, S], BF16, kind="ExternalInput").ap()
    wqt = nc.dram_tensor("wqt", [P, DC, 256], BF16, kind="ExternalInput").ap()
    wkt = nc.dram_tensor("wkt", [P, DC, 256], BF16, kind="ExternalInput").ap()
    wvt = nc.dram_tensor("wvt", [P, DC, 256], BF16, kind="ExternalInput").ap()
    # Wo pair-packed: partitions 0:64 = even unit of the pair, 64:128 = odd
    # unit, so the output projection contracts K=128 per matmul
    wot = nc.dram_tensor("wot", [P, PAIRS, D], F32R, kind="ExternalInput").ap()
    y = nc.dram_tensor("y", [S, D], F32, kind="ExternalOutput").ap()

    with tile.TileContext(nc) as tc, ExitStack() as ctx:
        persist = ctx.enter_context(tc.tile_pool(name="persist", bufs=1))
        pt_pool = ctx.enter_context(
            tc.tile_pool(name="pt", bufs=int(os.environ.get("K_PT", "4")))
        )
        otsb_pool = ctx.enter_context(
            tc.tile_pool(name="otsb", bufs=int(os.environ.get("K_OTSB", "4")))
        )
        rc_pool = ctx.enter_context(tc.tile_pool(name="rc", bufs=2))
        bc_pool = ctx.enter_context(tc.tile_pool(name="bc", bufs=2))
        y_pool = ctx.enter_context(
            tc.tile_pool(name="ysb", bufs=int(os.environ.get("K_Y", "4")))
        )
        st_ps = ctx.enter_context(
            tc.tile_pool(name="stps", bufs=int(os.environ.get("K_ST", "2")), space="PSUM")
        )
        ot_ps = ctx.enter_context(
            tc.tile_pool(name="otps", bufs=int(os.environ.get("K_OT", "2")), space="PSUM")
        )
        cell_ps = ctx.enter_context(
            tc.tile_pool(name="cellps", bufs=int(os.environ.get("K_CELL", "2")), space="PSUM")
        )

        # ---- persistent SBUF tensors ----
        KT = persist.tile([P, PAIRS, S], F32R)  # K^T pair-packed
        QT = persist.tile([P, PAIRS, S], F32R)  # Q^T pair-packed
        # V' per (k-chunk, pair): [V_unitA(64) | 1 | V_unitB(64) | 1];
        # each unit's PV is M=65 at base 0: out rows 0-63, sum at row 64
        Vp = persist.tile([P, KI, PAIRS, 130], F32R)
        wq_sb = persist.tile([P, DC, 256], BF16)
        wk_sb = persist.tile([P, DC, 256], BF16)
        wv_a = persist.tile([P, 4, 256], BF16)
        wv_b = persist.tile([P, 4, 256], BF16)
        wo_sb = persist.tile([P, PAIRS, D], F32R)  # pair-packed Wo rows (K=128)
        # O' accumulator: rows 0-63 unnormalized out^T, row 64 = denominator
        acc = persist.tile([65, QB, PAIRS, 2, 512], F32)
        # hidden^T lives only within its round
        ht_pool = ctx.enter_context(
            tc.tile_pool(name="htp", bufs=int(os.environ.get("K_HT", "2")))
        )

        # memset can't write fp32r; stage ones in f32 and round via copies
        ones_f32 = persist.tile([P, P], F32)
        nc.vector.memset(ones_f32, 1.0)
        # weights go on the SWDGE queue so they don't head-block the
        # hidden-tile loads on the HWDGE queue; ordered by first use: V
        # projection runs first in round 0, Wo isn't needed until the
        # first finalize
        nc.gpsimd.dma_start(wv_a, wvt[:, 0:4, :])
        nc.gpsimd.dma_start(wv_b, wvt[:, 4:8, :])
        nc.gpsimd.dma_start(wk_sb, wkt)
        nc.gpsimd.dma_start(wq_sb, wqt)
        nc.gpsimd.dma_start(wo_sb, wot)
        for col in (64, 129):
            nc.vector.tensor_copy(
                Vp[:, :, :, col : col + 1],
                ones_f32[:, 0:32].rearrange("p (a b c) -> p a b c", a=KI, b=PAIRS),
            )

        def cell_pair(qb, kc, p):
            """One head-pair of the attention cell (qb, kc)."""
            otA = cell_ps.tile([P, 512], F32, tag="ot", name="otA")
            otB = cell_ps.tile([P, 512], F32, tag="ot", name="otB")
            for k4 in range(4):
                ki = kc * 4 + k4
                stt = st_ps.tile([P, 1024], F32, tag="st")
                nc.tensor.matmul(
                    stt[:, 0:512],
                    KT[0:64, p, ts(ki, 128)],
                    QT[0:64, p, ts(qb, 512)],
                    start=True,
                    stop=True,
                )
                nc.tensor.matmul(
                    stt[:, 512:1024],
                    KT[64:128, p, ts(ki, 128)],
                    QT[64:128, p, ts(qb, 512)],
                    start=True,
                    stop=True,
                )
                pt = pt_pool.tile([P, 1024], F32R)
                nc.scalar.activation(
                    pt, stt, mybir.ActivationFunctionType.Exp, scale=SCALE
                )
                nc.tensor.matmul(
                    otA[0:65, :],
                    Vp[:, ki, p, 0:65],
                    pt[:, 0:512],
                    start=(k4 == 0),
                    stop=(k4 == 3),
                )
                nc.tensor.matmul(
                    otB[0:65, :],
                    Vp[:, ki, p, 65:130],
                    pt[:, 512:1024],
                    start=(k4 == 0),
                    stop=(k4 == 3),
                )
            for u, ot in ((0, otA), (1, otB)):
                sl = acc[:, qb, p, u, :]
                if kc == 0:
                    nc.vector.tensor_copy(sl, ot[0:65, :])
                else:
                    nc.vector.tensor_add(sl, sl, ot[0:65, :])

        ot_pairs = {}  # (qb, p) -> normalized pair-packed O^T tile

        def normalize(qb, p):
            """Reciprocal + broadcast + scale for one (q-block, pair).

            DVE/Pool only — contributes no PE work, so it can be emitted
            immediately after the pair's last cell without stalling the PE.
            """
            # one batched reciprocal covers both units of the pair; the
            # result lands at partition 0 — the hardware partition
            # broadcast does not honor a partition-offset input
            rc = rc_pool.tile([1, 1024], F32R)
            with nc.allow_low_precision(
                reason="fp32r rounding of softmax scale is plenty"
            ):
                nc.vector.reciprocal(
                    rc,
                    acc[64:65, qb, p, :, :].rearrange("p a f -> p (a f)"),
                )
            # broadcast 1/denom across partitions on the idle Pool engine
            bc = bc_pool.tile([64, 1024], F32R)
            nc.gpsimd.partition_broadcast(bc, rc)
            # pair-packed normalized O^T: partitions 0:64 = even unit,
            # 64:128 = odd unit, so the output projection gets K=128; the
            # multiplies are SBUF-only so they run on Pool, keeping DVE
            # free for the PSUM-side copies it alone (with ACT) can do
            otp = otsb_pool.tile([P, 512], F32R)
            nc.gpsimd.tensor_mul(otp[0:64, :], acc[0:64, qb, p, 0, :], bc[:, 0:512])
            nc.gpsimd.tensor_mul(otp[64:128, :], acc[0:64, qb, p, 1, :], bc[:, 512:1024])
            ot_pairs[(qb, p)] = otp

        def outproj(qb, groups, act_period=4, pools=None, pool_dma=False):
            """Output projection for q-block qb over qt_i groups.

            Each qt_i covers the full 1024-wide output row block as two
            oh-half matmuls into separate PSUM tiles whose copies land in
            one [128, 1024] SBUF tile, stored with a single DMA (halving
            the per-store engine push cost).
            """
            if pools is None:
                pools = [ot_ps]
            for gi, qt_i in enumerate(groups):
                ysb = y_pool.tile([P, 1024], F32)
                for oh in range(2):
                    # once the attention cells are done their PSUM pool
                    # joins the rotation, doubling copy/matmul overlap
                    yps = pools[(2 * gi + oh) % len(pools)].tile(
                        [P, 512], F32, tag="ot"
                    )
                    for p in range(PAIRS):
                        nc.tensor.matmul(
                            yps,
                            ot_pairs[(qb, p)][:, ts(qt_i, 128)],
                            wo_sb[:, p, ds(oh * 512, 512)],
                            start=(p == 0),
                            stop=(p == PAIRS - 1),
                        )
                    # spill some PSUM->SBUF copies to ACT (Copy shares
                    # Exp's activation table - no reload) so DVE doesn't
                    # pace the final stretch; ACT still carries remaining
                    # exps, so only every act_period-th copy goes there
                    if (qt_i * 2 + oh) % act_period == act_period - 1:
                        nc.scalar.activation(
                            ysb[:, ds(oh * 512, 512)],
                            yps,
                            mybir.ActivationFunctionType.Copy,
                            scale=1.0,
                        )
                    else:
                        nc.vector.tensor_copy(ysb[:, ds(oh * 512, 512)], yps)
                rows = y[qb * 512 + qt_i * 128 : qb * 512 + (qt_i + 1) * 128, :]
                # alternate the store between the SP (HWDGE) and Pool
                # (SWDGE) queues in the final stretch so the pushes don't
                # serialize on one engine behind the last matmuls
                if pool_dma and gi % 2 == 1:
                    nc.gpsimd.dma_start(rows, ysb)
                else:
                    nc.sync.dma_start(rows, ysb)

        # ---- pipeline: per s-block round, DMA + project. The in-order PE
        # needs ACT-independent filler while the exp chain of a cell-pair
        # (~4.2us ACT vs ~3.4us PE) catches up, so cell-pairs are woven
        # between projection chains: round r emits its Q chains first
        # (woven with pairs carried from round r-1), then V/K chains woven
        # with round r's own (r, kc<r) pairs (those only need Q_r), and
        # carries the (qb<=r, r) pairs into round r+1's Q phase. ----
        def interleave(chains, pairs):
            """Emit chain thunks with pair thunks distributed evenly."""
            n_c, n_p = len(chains), len(pairs)
            if n_c == 0:
                for t in pairs:
                    t()
                return
            # pairs after chain i: spread n_p slots over n_c positions
            emitted = 0
            for i, ch in enumerate(chains):
                ch()
                want = (i + 1) * n_p // n_c
                while emitted < want:
                    pairs[emitted]()
                    emitted += 1

        carry = []  # cell-pair thunks carried from the previous round
        for sb in range(SB):
            bounds = [0, 1, 4, 6, 8] if sb == 0 else [0, 2, 4, 6, 8]
            hT_tiles = []
            for i in range(4):
                lo, hi = bounds[i], bounds[i + 1]
                htt = ht_pool.tile([P, hi - lo, 512], BF16, tag=f"ht{i}", name=f"ht{i}")
                hT_tiles.append((htt, lo, hi))
            for t, lo, hi in hT_tiles:
                nc.sync.dma_start(t, hiddent[:, lo:hi, ts(sb, 512)])

            def hts(dc, hT_tiles=hT_tiles):
                for t, lo, hi in hT_tiles:
                    if lo <= dc < hi:
                        return t[:, dc - lo, :]
                raise AssertionError(dc)

            def v_chain(st, sb=sb, hts=hts):
                ki_idx = sb * 4 + st
                vps = ot_ps.tile([P, 512], F32, tag="ot", name="vps")
                for dc in range(DC):
                    nc.tensor.matmul(
                        vps[:, :256],
                        hts(dc)[:, ts(st, 128)],
                        (wv_a if dc < 4 else wv_b)[:, dc % 4, :],
                        start=(dc == 0),
                        stop=(dc == DC - 1),
                    )
                for p in range(PAIRS):
                    nc.vector.tensor_copy(
                        Vp[:, ki_idx, p, 0:64], vps[:, (2 * p) * 64 : (2 * p + 1) * 64]
                    )
                    nc.vector.tensor_copy(
                        Vp[:, ki_idx, p, 65:129],
                        vps[:, (2 * p + 1) * 64 : (2 * p + 2) * 64],
                    )

            def kq_chain(w_sb, out_t, p, sb=sb, hts=hts):
                kps = ot_ps.tile([P, 512], F32, tag="ot", name="kps")
                for dc in range(DC):
                    nc.tensor.matmul(
                        kps,
                        w_sb[:, dc, ts(p, 128)],
                        hts(dc),
                        start=(dc == 0),
                        stop=(dc == DC - 1),
                    )
                nc.vector.tensor_copy(out_t[:, p, ts(sb, 512)], kps)

            q_chains = [lambda p=p: kq_chain(wq_sb, QT, p) for p in range(PAIRS)]
            vk_chains = [lambda st=st: v_chain(st) for st in range(4)]
            vk_chains += [lambda p=p: kq_chain(wk_sb, KT, p) for p in range(PAIRS)]
            # this round's own early pairs: q-block sb against keys kc < sb
            # (they need only Q_sb from this round's chains)
            q_pairs = [
                lambda kc=kc, p=p: cell_pair(sb, kc, p)
                for kc in range(sb)
                for p in range(PAIRS)
            ]
            if sb == 0:
                # round 0: V/K first (smaller wv_a lands first so the PE
                # starts earlier off the cold DMA), Q at the end
                for ch in vk_chains + q_chains:
                    ch()
            else:
                n_q = min(len(carry), max(1, (len(carry) + len(q_pairs)) // 4))
                interleave(q_chains, carry[:n_q])
                interleave(vk_chains, carry[n_q:] + q_pairs)
            # carry (qb, sb) for qb <= sb into the next round's Q phase
            carry = [
                lambda qb=qb, kc=sb, p=p: cell_pair(qb, kc, p)
                for qb in list(range(sb)) + [sb]
                for p in range(PAIRS)
            ]

        # ---- final stretch: the (qb, 3) pairs, with normalizes emitted
        # per-pair as their data lands and output projections split in
        # halves to fill the remaining cells' ACT-wait gaps; q-block 3
        # first so its finalize unlocks the first filler ----
        g_lo, g_hi = [0, 1], [2, 3]
        final_cells = [(3, 3), (0, 3), (1, 3), (2, 3)]  # (3,3) leads via carry
        post = {  # after pair (qb, kc, p) emit these filler actions
            (3, 3, 0): [lambda: normalize(3, 0)],
            (3, 3, 1): [lambda: normalize(3, 1)],
            (0, 3, 0): [lambda: normalize(0, 0), lambda: outproj(3, g_lo)],
            (0, 3, 1): [lambda: normalize(0, 1), lambda: outproj(3, g_hi)],
            (1, 3, 0): [lambda: normalize(1, 0), lambda: outproj(0, g_lo)],
            (1, 3, 1): [lambda: normalize(1, 1), lambda: outproj(0, g_hi)],
            (2, 3, 0): [lambda: normalize(2, 0), lambda: outproj(1, g_lo)],
            (2, 3, 1): [
                lambda: normalize(2, 1),
                lambda: outproj(1, g_hi, act_period=2, pools=[ot_ps, cell_ps]),
            ],
        }
        # `carry` out of round 3 is exactly the final_cells' pair thunks;
        # re-emit them here with the filler weave instead
        for qb, kc in final_cells:
            for p in range(PAIRS):
                cell_pair(qb, kc, p)
                for act in post.get((qb, kc, p), []):
                    act()
        outproj(
            2, g_lo + g_hi, act_period=2, pools=[ot_ps, cell_ps], pool_dma=True
        )
    nc.compile()
    return nc


_NC = None


def get_nc():
    global _NC
    if _NC is None:
        _NC = build_nc()
    return _NC


def shard_inputs(hidden_states, Wq, Wk, Wv, Wo):
    """Per-core input maps. Core c: batch c//4, heads 4*(c%4) .. 4*(c%4)+3."""
    import ml_dtypes

    bf16 = ml_dtypes.bfloat16
    hidden_states = np.asarray(hidden_states, np.float32)
    Wq, Wk, Wv, Wo = (np.asarray(w, np.float32) for w in (Wq, Wk, Wv, Wo))
    in_maps = []
    for c in range(N_CORES):
        b = c // 4
        f0 = (c % 4) * 4 * DIM_HEAD  # first feature row/col of this core's heads
        rows = slice(f0, f0 + UNITS * DIM_HEAD)

        def proj_layout(w):
            # W[rows].T is [D, 256]; on-chip layout is [128, DC, 256]
            return np.ascontiguousarray(
                w[rows, :].T.reshape(DC, P, 256).transpose(1, 0, 2).astype(bf16)
            )

        # Wo[:, rows].T is [256, D]; pair-packed on-chip layout [128, PAIRS, D]
        wot = np.ascontiguousarray(
            Wo[:, rows].T.reshape(PAIRS, P, D).transpose(1, 0, 2)
        )
        in_maps.append(
            {
                "hiddent": np.ascontiguousarray(
                    hidden_states[b].T.reshape(DC, P, S).transpose(1, 0, 2).astype(bf16)
                ),
                "wqt": proj_layout(Wq),
                "wkt": proj_layout(Wk),
                "wvt": proj_layout(Wv),
                "wot": wot,
            }
        )
    return in_maps


def unshard_outputs(results, bo):
    out = np.zeros((B, S, D), np.float32)
    for c, res in enumerate(results):
        out[c // 4] += res["y"]
    out += np.asarray(bo, np.float32)[None, None, :]
    return out


def kernel(hidden_states, Wq, Wk, Wv, Wo, bo, _trace=False):
    from concourse.bass_utils import run_bass_kernel_spmd

    nc = get_nc()
    in_maps = shard_inputs(hidden_states, Wq, Wk, Wv, Wo)
    res = run_bass_kernel_spmd(nc, in_maps, list(range(N_CORES)), trace=_trace)
    out = unshard_outputs(res.results, bo)
    if _trace:
        return out, res
    return out


# revision 40
# speedup vs baseline: 1.1550x; 1.0125x over previous
# CrossAttention (B=2, S=2048, D=1024, H=16, dh=64) on 8 trn2 NeuronCores.
#
# Sharding: 32 (batch, head) units, 4 consecutive units per core (cores 0-3
# work on batch 0, cores 4-7 on batch 1). Each core receives its batch's
# hidden states pre-permuted to the on-chip [128, D/128, S] transposed
# layout, per-head slices of Wq/Wk/Wv/Wo (also pre-permuted); it returns a
# partial output y [2048, 1024] (its heads' contribution to the output
# projection). The host sums the four partials per batch and adds bo
# (tensor-parallel unshard of the output projection).
#
# Device algorithm (per core, 4 units = 2 pairs of heads), all matmuls
# fp32r (full PE rate at N>=256, fp32 accumulation).
#
# Schedule: the PE executes in order, so ACT-bound attention cells (the
# exp chain of a cell-pair takes ~4.2us vs ~3.4us of PE work) are woven
# between PE-only projection chains at pair granularity: every round's
# cells are deferred one round and interleaved with the next round's
# V/K/Q projection chains, keeping the PE fed while ACT catches up. The
# last round's cells interleave with the finalizes (normalize + output
# projection), whose PE work plays the same filler role.
#
# A cell (qb, kc), per pair: 4 key chunks of S^T = K^T-chunk^T Q^T (two
# heads row-packed, K=64 each), P^T = exp(S^T/8) on ACT (one [128,1024]
# op covers both heads), O' += V'^T P^T (M=65: psum rows 0-63
# unnormalized out^T, row 64 = softmax denominator via a fused ones
# column in V'); O' accumulates across rounds in SBUF.
#
# Finalize per q-block: DVE reciprocal of the denominators (written to
# partition 0), GPSIMD partition-broadcast on the otherwise-idle Pool
# engine, DVE multiply into a [128, 512] pair-packed tile (partitions
# 0:64 even unit, 64:128 odd unit), then the output projection contracts
# K=128 per matmul; PSUM->SBUF output copies alternate DVE/ACT so neither
# engine paces the tail, and the final store is split in half so the
# last DMA transfer off the critical path is short.
#
# PSUM plan (8 banks): S^T tiles 2x[128,1024] (4 banks), PV accumulators
# 2x[128,512] (2 banks), projection/output transients 2x[128,512] (2
# banks) - dedicated pools so the streams don't steal each other's slots.
import os
import sys

import numpy as np

try:
    import concourse.bass as bass
except ImportError:  # harness runs from a fresh dir; repo is on the default path
    sys.path.insert(0, "/opt/trn_rl_repo")
    import concourse.bass as bass

import concourse.bacc as bacc
import concourse.mybir as mybir
import concourse.tile as tile
from concourse.bass import ts, ds
from contextlib import ExitStack

B, S, D = 2, 2048, 1024
HEADS, DIM_HEAD = 16, 64
SCALE = DIM_HEAD**-0.5
N_CORES = 8
UNITS = 4  # (b, h) units per core
PAIRS = 2  # head pairs per core
P = 128
SB = S // 512  # 4 s-blocks of 512
QB = S // 512  # 4 q-blocks of 512
DC = D // P  # 8 contraction chunks for projections
KI = S // P  # 16 key chunks of 128
F32 = mybir.dt.float32
F32R = mybir.dt.float32r
BF16 = mybir.dt.bfloat16


def build_nc():
    nc = bacc.Bacc("TRN2", target_bir_lowering=False, debug=False)

    # hidden and the QKV projection weights ship as bf16: the projection
    # matmuls are pure bf16 x bf16 (full PE rate, fp32 accumulation) and
    # the input DMA bytes halve, which is what paces the cold start
    hiddent = nc.dram_tensor("hiddent", [P, DC, S], BF16, kind="ExternalInput").ap()
    wqt = nc.dram_tensor("wqt", [P, DC, 256], BF16, kind="ExternalInput").ap()
    wkt = nc.dram_tensor("wkt", [P, DC, 256], BF16, kind="ExternalInput").ap()
    wvt = nc.dram_tensor("wvt", [P, DC, 256], BF16, kind="ExternalInput").ap()
    # Wo pair-packed: partitions 0:64 = even unit of the pair, 64:128 = odd
    # unit, so the output projection contracts K=128 per matmul
    wot = nc.dram_tensor("wot", [P, PAIRS, D], F32R, kind="ExternalInput").ap()
    y = nc.dram_tensor("y", [S, D], F32, kind="ExternalOutput").ap()

    with tile.TileContext(nc) as tc, ExitStack() as ctx:
        persist = ctx.enter_context(tc.tile_pool(name="persist", bufs=1))
        pt_pool = ctx.enter_context(
            tc.tile_pool(name="pt", bufs=int(os.environ.get("K_PT", "4")))
        )
        otsb_pool = ctx.enter_context(
            tc.tile_pool(name="otsb", bufs=int(os.environ.get("K_OTSB", "4")))
        )
        rc_pool = ctx.enter_context(tc.tile_pool(name="rc", bufs=2))
        bc_pool = ctx.enter_context(tc.tile_pool(name="bc", bufs=2))
        y_pool = ctx.enter_context(
            tc.tile_pool(name="ysb", bufs=int(os.environ.get("K_Y", "8")))
        )
        st_ps = ctx.enter_context(
            tc.tile_pool(name="stps", bufs=int(os.environ.get("K_ST", "2")), space="PSUM")
        )
        ot_ps = ctx.enter_context(
            tc.tile_pool(name="otps", bufs=int(os.environ.get("K_OT", "2")), space="PSUM")
        )
        cell_ps = ctx.enter_context(
            tc.tile_pool(name="cellps", bufs=int(os.environ.get("K_CELL", "2")), space="PSUM")
        )

        # ---- persistent SBUF tensors ----
        KT = persist.tile([P, PAIRS, S], F32R)  # K^T pair-packed
        QT = persist.tile([P, PAIRS, S], F32R)  # Q^T pair-packed
        # V' per (k-chunk, pair): [V_unitA(64) | 1 | V_unitB(64) | 1];
        # each unit's PV is M=65 at base 0: out rows 0-63, sum at row 64
        Vp = persist.tile([P, KI, PAIRS, 130], F32R)
        wq_sb = persist.tile([P, DC, 256], BF16)
        wk_sb = persist.tile([P, DC, 256], BF16)
        wv_a = persist.tile([P, 4, 256], BF16)
        wv_b = persist.tile([P, 4, 256], BF16)
        wo_sb = persist.tile([P, PAIRS, D], F32R)  # pair-packed Wo rows (K=128)
        # O' accumulator: rows 0-63 unnormalized out^T, row 64 = denominator
        acc = persist.tile([65, QB, PAIRS, 2, 512], F32)
        # hidden^T lives only within its round
        ht_pool = ctx.enter_context(
            tc.tile_pool(name="htp", bufs=int(os.environ.get("K_HT", "2")))
        )

        # memset can't write fp32r; stage ones in f32 and round via copies
        ones_f32 = persist.tile([P, P], F32)
        nc.vector.memset(ones_f32, 1.0)
        # weights go on the SWDGE queue so they don't head-block the
        # hidden-tile loads on the HWDGE queue; ordered by first use: V
        # projection runs first in round 0, Wo isn't needed until the
        # first finalize
        nc.gpsimd.dma_start(wv_a, wvt[:, 0:4, :])
        nc.gpsimd.dma_start(wv_b, wvt[:, 4:8, :])
        nc.gpsimd.dma_start(wk_sb, wkt)
        nc.gpsimd.dma_start(wq_sb, wqt)
        nc.gpsimd.dma_start(wo_sb, wot)
        for col in (64, 129):
            nc.vector.tensor_copy(
                Vp[:, :, :, col : col + 1],
                ones_f32[:, 0:32].rearrange("p (a b c) -> p a b c", a=KI, b=PAIRS),
            )

        def cell_pair(qb, kc, p):
            """One head-pair of the attention cell (qb, kc)."""
            otA = cell_ps.tile([P, 512], F32, tag="ot", name="otA")
            otB = cell_ps.tile([P, 512], F32, tag="ot", name="otB")
            for k4 in range(4):
                ki = kc * 4 + k4
                stt = st_ps.tile([P, 1024], F32, tag="st")
                nc.tensor.matmul(
                    stt[:, 0:512],
                    KT[0:64, p, ts(ki, 128)],
                    QT[0:64, p, ts(qb, 512)],
                    start=True,
                    stop=True,
                )
                nc.tensor.matmul(
                    stt[:, 512:1024],
                    KT[64:128, p, ts(ki, 128)],
                    QT[64:128, p, ts(qb, 512)],
                    start=True,
                    stop=True,
                )
                pt = pt_pool.tile([P, 1024], F32R)
                nc.scalar.activation(
                    pt, stt, mybir.ActivationFunctionType.Exp, scale=SCALE
                )
                nc.tensor.matmul(
                    otA[0:65, :],
                    Vp[:, ki, p, 0:65],
                    pt[:, 0:512],
                    start=(k4 == 0),
                    stop=(k4 == 3),
                )
                nc.tensor.matmul(
                    otB[0:65, :],
                    Vp[:, ki, p, 65:130],
                    pt[:, 512:1024],
                    start=(k4 == 0),
                    stop=(k4 == 3),
                )
            for u, ot in ((0, otA), (1, otB)):
                sl = acc[:, qb, p, u, :]
                if kc == 0:
                    nc.vector.tensor_copy(sl, ot[0:65, :])
                else:
                    nc.vector.tensor_add(sl, sl, ot[0:65, :])

        ot_pairs = {}  # (qb, p) -> normalized pair-packed O^T tile

        def normalize(qb, p, per_unit=False):
            """Reciprocal + broadcast + scale for one (q-block, pair).

            DVE/Pool only — contributes no PE work, so it can be emitted
            immediately after the pair's last cell without stalling the PE.
            per_unit splits the chain so the first unit's scale completes
            without waiting for the second unit's accumulator — used for
            the very last pair, where this chain is the critical path.
            """
            # the reciprocal lands at partition 0 — the hardware partition
            # broadcast does not honor a partition-offset input
            otp = otsb_pool.tile([P, 512], F32R)
            with nc.allow_low_precision(
                reason="fp32r rounding of softmax scale is plenty"
            ):
                if per_unit:
                    for u in range(2):
                        rcu = rc_pool.tile([1, 512], F32R, tag="rcu", name="rcu")
                        nc.vector.reciprocal(rcu, acc[64:65, qb, p, u, :])
                        bcu = bc_pool.tile([64, 512], F32R, tag="bcu", name="bcu")
                        nc.gpsimd.partition_broadcast(bcu, rcu)
                        nc.gpsimd.tensor_mul(
                            otp[u * 64 : (u + 1) * 64, :], acc[0:64, qb, p, u, :], bcu
                        )
                    ot_pairs[(qb, p)] = otp
                    return
                # one batched reciprocal covers both units of the pair
                rc = rc_pool.tile([1, 1024], F32R)
                nc.vector.reciprocal(
                    rc,
                    acc[64:65, qb, p, :, :].rearrange("p a f -> p (a f)"),
                )
            # broadcast 1/denom across partitions on the idle Pool engine
            bc = bc_pool.tile([64, 1024], F32R)
            nc.gpsimd.partition_broadcast(bc, rc)
            # pair-packed normalized O^T: partitions 0:64 = even unit,
            # 64:128 = odd unit, so the output projection gets K=128; the
            # multiplies are SBUF-only so they run on Pool, keeping DVE
            # free for the PSUM-side copies it alone (with ACT) can do
            nc.gpsimd.tensor_mul(otp[0:64, :], acc[0:64, qb, p, 0, :], bc[:, 0:512])
            nc.gpsimd.tensor_mul(otp[64:128, :], acc[0:64, qb, p, 1, :], bc[:, 512:1024])
            ot_pairs[(qb, p)] = otp

        def outproj(qb, groups, act_period=4, pools=None, last_small=False):
            """Output projection for q-block qb over (qt_i, oh) groups.

            A DMA occupies its issuing engine for the whole transfer, so
            the stores alternate between the SP and Pool queues, and in
            the final stretch the PSUM->SBUF copies alternate DVE/ACT
            (Copy shares Exp's activation table - no reload) — four
            engines carry the drain in parallel.
            """
            if pools is None:
                pools = [ot_ps]
            for gi, (qt_i, oh) in enumerate(groups):
                # once the attention cells are done their PSUM pool joins
                # the rotation, doubling the copy/matmul overlap depth
                yps = pools[gi % len(pools)].tile([P, 512], F32, tag="ot")
                for p in range(PAIRS):
                    nc.tensor.matmul(
                        yps,
                        ot_pairs[(qb, p)][:, ts(qt_i, 128)],
                        wo_sb[:, p, ds(oh * 512, 512)],
                        start=(p == 0),
                        stop=(p == PAIRS - 1),
                    )
                ysb = y_pool.tile([P, 512], F32)
                rows = y[qb * 512 + qt_i * 128 : qb * 512 + (qt_i + 1) * 128, :]
                if last_small and gi == len(groups) - 1:
                    # very last store: halve the copy (DVE + ACT in
                    # parallel) and the store (SP + Pool in parallel) so
                    # the kernel-ending DMA latency chain is as short as
                    # possible
                    nc.vector.tensor_copy(ysb[:, 0:256], yps[:, 0:256])
                    nc.scalar.activation(
                        ysb[:, 256:512],
                        yps[:, 256:512],
                        mybir.ActivationFunctionType.Copy,
                        scale=1.0,
                    )
                    nc.sync.dma_start(rows[:, ds(oh * 512, 256)], ysb[:, 0:256])
                    nc.gpsimd.dma_start(
                        rows[:, ds(oh * 512 + 256, 256)], ysb[:, 256:512]
                    )
                    continue
                if (qt_i * 2 + oh) % act_period == act_period - 1:
                    nc.scalar.activation(
                        ysb, yps, mybir.ActivationFunctionType.Copy, scale=1.0
                    )
                else:
                    nc.vector.tensor_copy(ysb, yps)
                if (qt_i * 2 + oh) % 2 == 1:
                    nc.gpsimd.dma_start(rows[:, ds(oh * 512, 512)], ysb)
                else:
                    nc.sync.dma_start(rows[:, ds(oh * 512, 512)], ysb)

        # ---- pipeline: per s-block round, DMA + project. The in-order PE
        # needs ACT-independent filler while the exp chain of a cell-pair
        # (~4.2us ACT vs ~3.4us PE) catches up, so cell-pairs are woven
        # between projection chains: round r emits its Q chains first
        # (woven with pairs carried from round r-1), then V/K chains woven
        # with round r's own (r, kc<r) pairs (those only need Q_r), and
        # carries the (qb<=r, r) pairs into round r+1's Q phase. ----
        def interleave(chains, pairs):
            """Emit chain thunks with pair thunks distributed by weight.

            chains is a list of (thunk, weight) where weight ~ the chain's
            PE duration; pairs are spread proportionally so a long K/Q
            chain absorbs more ACT-deficit than a short V chain.
            """
            n_p = len(pairs)
            if not chains:
                for t in pairs:
                    t()
                return
            total_w = sum(w for _, w in chains)
            emitted, cum_w = 0, 0
            for ch, w in chains:
                ch()
                cum_w += w
                want = round(cum_w * n_p / total_w)
                while emitted < want:
                    pairs[emitted]()
                    emitted += 1
            while emitted < n_p:
                pairs[emitted]()
                emitted += 1

        carry = []  # cell-pair thunks carried from the previous round
        for sb in range(SB):
            bounds = [0, 1, 4, 6, 8] if sb == 0 else [0, 2, 4, 6, 8]
            hT_tiles = []
            for i in range(4):
                lo, hi = bounds[i], bounds[i + 1]
                htt = ht_pool.tile([P, hi - lo, 512], BF16, tag=f"ht{i}", name=f"ht{i}")
                hT_tiles.append((htt, lo, hi))
            for t, lo, hi in hT_tiles:
                nc.sync.dma_start(t, hiddent[:, lo:hi, ts(sb, 512)])

            def hts(dc, hT_tiles=hT_tiles):
                for t, lo, hi in hT_tiles:
                    if lo <= dc < hi:
                        return t[:, dc - lo, :]
                raise AssertionError(dc)

            def v_chain(st, sb=sb, hts=hts):
                ki_idx = sb * 4 + st
                vps = ot_ps.tile([P, 512], F32, tag="ot", name="vps")
                for dc in range(DC):
                    nc.tensor.matmul(
                        vps[:, :256],
                        hts(dc)[:, ts(st, 128)],
                        (wv_a if dc < 4 else wv_b)[:, dc % 4, :],
                        start=(dc == 0),
                        stop=(dc == DC - 1),
                    )
                for p in range(PAIRS):
                    nc.vector.tensor_copy(
                        Vp[:, ki_idx, p, 0:64], vps[:, (2 * p) * 64 : (2 * p + 1) * 64]
                    )
                    nc.vector.tensor_copy(
                        Vp[:, ki_idx, p, 65:129],
                        vps[:, (2 * p + 1) * 64 : (2 * p + 2) * 64],
                    )

            def kq_chain(w_sb, out_t, p, sb=sb, hts=hts):
                kps = ot_ps.tile([P, 512], F32, tag="ot", name="kps")
                for dc in range(DC):
                    nc.tensor.matmul(
                        kps,
                        w_sb[:, dc, ts(p, 128)],
                        hts(dc),
                        start=(dc == 0),
                        stop=(dc == DC - 1),
                    )
                nc.vector.tensor_copy(out_t[:, p, ts(sb, 512)], kps)

            q_chains = [
                (lambda p=p: kq_chain(wq_sb, QT, p), 2) for p in range(PAIRS)
            ]
            vk_chains = [(lambda st=st: v_chain(st), 1) for st in range(4)]
            vk_chains += [
                (lambda p=p: kq_chain(wk_sb, KT, p), 2) for p in range(PAIRS)
            ]
            # this round's own early pairs: q-block sb against keys kc < sb
            # (they need only Q_sb from this round's chains)
            q_pairs = [
                lambda kc=kc, p=p: cell_pair(sb, kc, p)
                for kc in range(sb)
                for p in range(PAIRS)
            ]
            # the diagonal cell (sb, sb) needs this round's V/K/Q; in the
            # last round weave it in at the end of the VK phase so the
            # final stretch has fewer filler-less pairs
            diag = [
                lambda p=p: cell_pair(sb, sb, p) for p in range(PAIRS)
            ] if sb == SB - 1 else []
            sched = os.environ.get("K_SCHED", "mixed")
            if sb == 0:
                # round 0: V/K first (smaller wv_a lands first so the PE
                # starts earlier off the cold DMA), Q at the end
                for ch, _ in vk_chains + q_chains:
                    ch()
            elif sched == "early":
                # chains first: later rounds' cells become available as
                # early as possible at the cost of ACT idling here
                for ch, _ in q_chains + vk_chains:
                    ch()
                for t in carry + q_pairs + diag:
                    t()
            elif sb == SB - 1 and sched == "mixed":
                # last round: V3's consumers are the diagonal and (qb, 3)
                # cells, so the V chains serve as filler deep in the
                # q_pairs stretch where the ACT deficit concentrates
                k_chains = [c for c in vk_chains if c[1] == 2]
                v_chains = [c for c in vk_chains if c[1] == 1]
                n_q = min(len(carry), max(1, (len(carry) + len(q_pairs)) // 3))
                interleave(q_chains, carry[:n_q])
                interleave(k_chains, carry[n_q:])
                interleave(v_chains, q_pairs)
                for t in diag:
                    t()
            else:
                n_q = min(len(carry), max(1, (len(carry) + len(q_pairs)) // 4))
                interleave(q_chains, carry[:n_q])
                interleave(vk_chains, carry[n_q:] + q_pairs)
                for t in diag:
                    t()
            # carry (qb, sb) for qb < sb (plus the diagonal except in the
            # last round) into the next round's Q phase
            carry = [
                lambda qb=qb, kc=sb, p=p: cell_pair(qb, kc, p)
                for qb in (list(range(sb)) + ([sb] if sb < SB - 1 else []))
                for p in range(PAIRS)
            ]

        # ---- final stretch: the (qb, 3) pairs, with normalizes emitted
        # per-pair as their data lands and output projections split in
        # halves to fill the remaining cells' ACT-wait gaps; q-block 3
        # first so its finalize unlocks the first filler ----
        g_lo = [(0, 0), (0, 1), (1, 0), (1, 1)]
        g_hi = [(2, 0), (2, 1), (3, 0), (3, 1)]
        # (3,3) already ran at the end of round 3's weave
        normalize(3, 0)
        normalize(3, 1)
        final_cells = [(0, 3), (1, 3), (2, 3)]
        post = {  # after pair (qb, kc, p) emit these filler actions
            (0, 3, 0): [lambda: normalize(0, 0), lambda: outproj(3, g_lo)],
            (0, 3, 1): [lambda: normalize(0, 1), lambda: outproj(3, g_hi)],
            (1, 3, 0): [lambda: normalize(1, 0), lambda: outproj(0, g_lo)],
            (1, 3, 1): [lambda: normalize(1, 1), lambda: outproj(0, g_hi)],
            (2, 3, 0): [lambda: normalize(2, 0)],
            (2, 3, 1): [
                lambda: normalize(2, 1, per_unit=True),
                # outproj(1) runs entirely after the last cell: it is the
                # PE filler that covers the last pair's normalize chain
                lambda: outproj(
                    1, g_lo + g_hi, act_period=2, pools=[ot_ps, cell_ps]
                ),
            ],
        }
        # `carry` out of round 3 is exactly the final_cells' pair thunks;
        # re-emit them here with the filler weave instead
        for qb, kc in final_cells:
            for p in range(PAIRS):
                cell_pair(qb, kc, p)
                for act in post.get((qb, kc, p), []):
                    act()
        outproj(2, g_lo + g_hi, act_period=2, pools=[ot_ps, cell_ps], last_small=True)
    nc.compile()
    return nc


_NC = None


def get_nc():
    global _NC
    if _NC is None:
        _NC = build_nc()
    return _NC


def shard_inputs(hidden_states, Wq, Wk, Wv, Wo):
    """Per-core input maps. Core c: batch c//4, heads 4*(c%4) .. 4*(c%4)+3."""
    import ml_dtypes

    bf16 = ml_dtypes.bfloat16
    hidden_states = np.asarray(hidden_states, np.float32)
    Wq, Wk, Wv, Wo = (np.asarray(w, np.float32) for w in (Wq, Wk, Wv, Wo))
    in_maps = []
    for c in range(N_CORES):
        b = c // 4
        f0 = (c % 4) * 4 * DIM_HEAD  # first feature row/col of this core's heads
        rows = slice(f0, f0 + UNITS * DIM_HEAD)

        def proj_layout(w):
            # W[rows].T is [D, 256]; on-chip layout is [128, DC, 256]
            return np.ascontiguousarray(
                w[rows, :].T.reshape(DC, P, 256).transpose(1, 0, 2).astype(bf16)
            )

        # Wo[:, rows].T is [256, D]; pair-packed on-chip layout [128, PAIRS, D]
        wot = np.ascontiguousarray(
            Wo[:, rows].T.reshape(PAIRS, P, D).transpose(1, 0, 2)
        )
        in_maps.append(
            {
                "hiddent": np.ascontiguousarray(
                    hidden_states[b].T.reshape(DC, P, S).transpose(1, 0, 2).astype(bf16)
                ),
                "wqt": proj_layout(Wq),
                "wkt": proj_layout(Wk),
                "wvt": proj_layout(Wv),
                "wot": wot,
            }
        )
    return in_maps


def unshard_outputs(results, bo):
    out = np.zeros((B, S, D), np.float32)
    for c, res in enumerate(results):
        out[c // 4] += res["y"]
    out += np.asarray(bo, np.float32)[None, None, :]
    return out


def kernel(hidden_states, Wq, Wk, Wv, Wo, bo, _trace=False):
    from concourse.bass_utils import run_bass_kernel_spmd

    nc = get_nc()
    in_maps = shard_inputs(hidden_states, Wq, Wk, Wv, Wo)
    res = run_bass_kernel_spmd(nc, in_maps, list(range(N_CORES)), trace=_trace)
    out = unshard_outputs(res.results, bo)
    if _trace:
        return out, res
    return out


# revision 52
# speedup vs baseline: 1.1602x; 1.0045x over previous
# CrossAttention (B=2, S=2048, D=1024, H=16, dh=64) on 8 trn2 NeuronCores.
#
# Sharding: 32 (batch, head) units, 4 consecutive units per core (cores 0-3
# work on batch 0, cores 4-7 on batch 1). Each core receives its batch's
# hidden states pre-permuted to the on-chip [128, D/128, S] transposed
# layout, per-head slices of Wq/Wk/Wv/Wo (also pre-permuted); it returns a
# partial output y [2048, 1024] (its heads' contribution to the output
# projection). The host sums the four partials per batch and adds bo
# (tensor-parallel unshard of the output projection).
#
# Device algorithm (per core, 4 units = 2 pairs of heads), all matmuls
# fp32r (full PE rate at N>=256, fp32 accumulation).
#
# Schedule: the PE executes in order, so ACT-bound attention cells (the
# exp chain of a cell-pair takes ~4.2us vs ~3.4us of PE work) are woven
# between PE-only projection chains at pair granularity: every round's
# cells are deferred one round and interleaved with the next round's
# V/K/Q projection chains, keeping the PE fed while ACT catches up. The
# last round's cells interleave with the finalizes (normalize + output
# projection), whose PE work plays the same filler role.
#
# A cell (qb, kc), per pair: 4 key chunks of S^T = K^T-chunk^T Q^T (two
# heads row-packed, K=64 each), P^T = exp(S^T/8) on ACT (one [128,1024]
# op covers both heads), O' += V'^T P^T (M=65: psum rows 0-63
# unnormalized out^T, row 64 = softmax denominator via a fused ones
# column in V'); O' accumulates across rounds in SBUF.
#
# Finalize per q-block: DVE reciprocal of the denominators (written to
# partition 0), GPSIMD partition-broadcast on the otherwise-idle Pool
# engine, DVE multiply into a [128, 512] pair-packed tile (partitions
# 0:64 even unit, 64:128 odd unit), then the output projection contracts
# K=128 per matmul; PSUM->SBUF output copies alternate DVE/ACT so neither
# engine paces the tail, and the final store is split in half so the
# last DMA transfer off the critical path is short.
#
# PSUM plan (8 banks): S^T tiles 2x[128,1024] (4 banks), PV accumulators
# 2x[128,512] (2 banks), projection/output transients 2x[128,512] (2
# banks) - dedicated pools so the streams don't steal each other's slots.
import os
import sys

import numpy as np

try:
    import concourse.bass as bass
except ImportError:  # harness runs from a fresh dir; repo is on the default path
    sys.path.insert(0, "/opt/trn_rl_repo")
    import concourse.bass as bass

import concourse.bacc as bacc
import concourse.mybir as mybir
import concourse.tile as tile
from concourse.bass import ts, ds
from contextlib import ExitStack

B, S, D = 2, 2048, 1024
HEADS, DIM_HEAD = 16, 64
SCALE = DIM_HEAD**-0.5
N_CORES = 8
UNITS = 4  # (b, h) units per core
PAIRS = 2  # head pairs per core
P = 128
SB = S // 512  # 4 s-blocks of 512
QB = S // 512  # 4 q-blocks of 512
DC = D // P  # 8 contraction chunks for projections
KI = S // P  # 16 key chunks of 128
F32 = mybir.dt.float32
F32R = mybir.dt.float32r
BF16 = mybir.dt.bfloat16


def build_nc():
    nc = bacc.Bacc("TRN2", target_bir_lowering=False, debug=False)

    # hidden and the QKV projection weights ship as bf16: the projection
    # matmuls are pure bf16 x bf16 (full PE rate, fp32 accumulation) and
    # the input DMA bytes halve, which is what paces the cold start
    hiddent = nc.dram_tensor("hiddent", [P, DC, S], BF16, kind="ExternalInput").ap()
    wqt = nc.dram_tensor("wqt", [P, DC, 256], BF16, kind="ExternalInput").ap()
    wkt = nc.dram_tensor("wkt", [P, DC, 256], BF16, kind="ExternalInput").ap()
    wvt = nc.dram_tensor("wvt", [P, DC, 256], BF16, kind="ExternalInput").ap()
    # Wo pair-packed: partitions 0:64 = even unit of the pair, 64:128 = odd
    # unit, so the output projection contracts K=128 per matmul
    wot = nc.dram_tensor("wot", [P, PAIRS, D], F32R, kind="ExternalInput").ap()
    y = nc.dram_tensor("y", [S, D], F32, kind="ExternalOutput").ap()

    with tile.TileContext(nc) as tc, ExitStack() as ctx:
        persist = ctx.enter_context(tc.tile_pool(name="persist", bufs=1))
        pt_pool = ctx.enter_context(
            tc.tile_pool(name="pt", bufs=int(os.environ.get("K_PT", "4")))
        )
        otsb_pool = ctx.enter_context(
            tc.tile_pool(name="otsb", bufs=int(os.environ.get("K_OTSB", "4")))
        )
        rc_pool = ctx.enter_context(tc.tile_pool(name="rc", bufs=2))
        bc_pool = ctx.enter_context(tc.tile_pool(name="bc", bufs=2))
        y_pool = ctx.enter_context(
            tc.tile_pool(name="ysb", bufs=int(os.environ.get("K_Y", "8")))
        )
        st_ps = ctx.enter_context(
            tc.tile_pool(name="stps", bufs=int(os.environ.get("K_ST", "2")), space="PSUM")
        )
        ot_ps = ctx.enter_context(
            tc.tile_pool(name="otps", bufs=int(os.environ.get("K_OT", "2")), space="PSUM")
        )
        cell_ps = ctx.enter_context(
            tc.tile_pool(name="cellps", bufs=int(os.environ.get("K_CELL", "2")), space="PSUM")
        )

        # ---- persistent SBUF tensors ----
        KT = persist.tile([P, PAIRS, S], F32R)  # K^T pair-packed
        QT = persist.tile([P, PAIRS, S], F32R)  # Q^T pair-packed
        # V' per (k-chunk, pair): [V_unitA(64) | 1 | V_unitB(64) | 1];
        # each unit's PV is M=65 at base 0: out rows 0-63, sum at row 64
        Vp = persist.tile([P, KI, PAIRS, 130], F32R)
        wq_sb = persist.tile([P, DC, 256], BF16)
        wk_sb = persist.tile([P, DC, 256], BF16)
        wv_a = persist.tile([P, 4, 256], BF16)
        wv_b = persist.tile([P, 4, 256], BF16)
        wo_sb = persist.tile([P, PAIRS, D], F32R)  # pair-packed Wo rows (K=128)
        # O' accumulator: rows 0-63 unnormalized out^T, row 64 = denominator
        acc = persist.tile([65, QB, PAIRS, 2, 512], F32)
        # hidden^T lives only within its round
        ht_pool = ctx.enter_context(
            tc.tile_pool(name="htp", bufs=int(os.environ.get("K_HT", "2")))
        )

        # memset can't write fp32r; stage ones in f32 and round via copies
        ones_f32 = persist.tile([P, P], F32)
        nc.vector.memset(ones_f32, 1.0)
        # weights go on the SWDGE queue so they don't head-block the
        # hidden-tile loads on the HWDGE queue; ordered by first use: V
        # projection runs first in round 0, Wo isn't needed until the
        # first finalize
        nc.gpsimd.dma_start(wv_a, wvt[:, 0:4, :])
        nc.gpsimd.dma_start(wv_b, wvt[:, 4:8, :])
        nc.gpsimd.dma_start(wk_sb, wkt)
        nc.gpsimd.dma_start(wq_sb, wqt)
        # wo (f32r, the biggest weight) is issued on the SP queue during
        # round 1 — it is not needed until the first finalize and would
        # otherwise delay wq behind it on the Pool queue
        for col in (64, 129):
            nc.vector.tensor_copy(
                Vp[:, :, :, col : col + 1],
                ones_f32[:, 0:32].rearrange("p (a b c) -> p a b c", a=KI, b=PAIRS),
            )

        def cell_pair(qb, kc, p, halves=(0, 1), state=None):
            """One head-pair of the attention cell (qb, kc).

            halves/state allow emitting the pair as two 2-k-chunk halves
            with filler chains woven between them (the PSUM accumulators
            live in `state` across the two calls).
            """
            if state is None:
                state = {}
            if 0 in halves:
                state["otA"] = cell_ps.tile([P, 512], F32, tag="ot", name="otA")
                state["otB"] = cell_ps.tile([P, 512], F32, tag="ot", name="otB")
            otA, otB = state["otA"], state["otB"]
            for k4 in [k for h in halves for k in (2 * h, 2 * h + 1)]:
                ki = kc * 4 + k4
                stt = st_ps.tile([P, 1024], F32, tag="st")
                nc.tensor.matmul(
                    stt[:, 0:512],
                    KT[0:64, p, ts(ki, 128)],
                    QT[0:64, p, ts(qb, 512)],
                    start=True,
                    stop=True,
                )
                nc.tensor.matmul(
                    stt[:, 512:1024],
                    KT[64:128, p, ts(ki, 128)],
                    QT[64:128, p, ts(qb, 512)],
                    start=True,
                    stop=True,
                )
                pt = pt_pool.tile([P, 1024], F32R)
                nc.scalar.activation(
                    pt, stt, mybir.ActivationFunctionType.Exp, scale=SCALE
                )
                nc.tensor.matmul(
                    otA[0:65, :],
                    Vp[:, ki, p, 0:65],
                    pt[:, 0:512],
                    start=(k4 == 0),
                    stop=(k4 == 3),
                )
                nc.tensor.matmul(
                    otB[0:65, :],
                    Vp[:, ki, p, 65:130],
                    pt[:, 512:1024],
                    start=(k4 == 0),
                    stop=(k4 == 3),
                )
            if 1 in halves:
                for u, ot in ((0, otA), (1, otB)):
                    sl = acc[:, qb, p, u, :]
                    if kc == 0:
                        nc.vector.tensor_copy(sl, ot[0:65, :])
                    else:
                        nc.vector.tensor_add(sl, sl, ot[0:65, :])

        ot_pairs = {}  # (qb, p) -> normalized pair-packed O^T tile

        def normalize(qb, p, per_unit=False):
            """Reciprocal + broadcast + scale for one (q-block, pair).

            DVE/Pool only — contributes no PE work, so it can be emitted
            immediately after the pair's last cell without stalling the PE.
            per_unit splits the chain so the first unit's scale completes
            without waiting for the second unit's accumulator — used for
            the very last pair, where this chain is the critical path.
            """
            # the reciprocal lands at partition 0 — the hardware partition
            # broadcast does not honor a partition-offset input
            otp = otsb_pool.tile([P, 512], F32R)
            with nc.allow_low_precision(
                reason="fp32r rounding of softmax scale is plenty"
            ):
                if per_unit:
                    for u in range(2):
                        rcu = rc_pool.tile([1, 512], F32R, tag="rcu", name="rcu")
                        nc.vector.reciprocal(rcu, acc[64:65, qb, p, u, :])
                        bcu = bc_pool.tile([64, 512], F32R, tag="bcu", name="bcu")
                        nc.gpsimd.partition_broadcast(bcu, rcu)
                        nc.gpsimd.tensor_mul(
                            otp[u * 64 : (u + 1) * 64, :], acc[0:64, qb, p, u, :], bcu
                        )
                    ot_pairs[(qb, p)] = otp
                    return
                # one batched reciprocal covers both units of the pair
                rc = rc_pool.tile([1, 1024], F32R)
                nc.vector.reciprocal(
                    rc,
                    acc[64:65, qb, p, :, :].rearrange("p a f -> p (a f)"),
                )
            # broadcast 1/denom across partitions on the idle Pool engine
            bc = bc_pool.tile([64, 1024], F32R)
            nc.gpsimd.partition_broadcast(bc, rc)
            # pair-packed normalized O^T: partitions 0:64 = even unit,
            # 64:128 = odd unit, so the output projection gets K=128; the
            # multiplies are SBUF-only so they run on Pool, keeping DVE
            # free for the PSUM-side copies it alone (with ACT) can do
            nc.gpsimd.tensor_mul(otp[0:64, :], acc[0:64, qb, p, 0, :], bc[:, 0:512])
            nc.gpsimd.tensor_mul(otp[64:128, :], acc[0:64, qb, p, 1, :], bc[:, 512:1024])
            ot_pairs[(qb, p)] = otp

        def outproj(qb, groups, act_period=4, pools=None, last_small=False):
            """Output projection for q-block qb over (qt_i, oh) groups.

            A DMA occupies its issuing engine for the whole transfer, so
            the stores alternate between the SP and Pool queues, and in
            the final stretch the PSUM->SBUF copies alternate DVE/ACT
            (Copy shares Exp's activation table - no reload) — four
            engines carry the drain in parallel.
            """
            if pools is None:
                pools = [ot_ps]
            for gi, (qt_i, oh) in enumerate(groups):
                # once the attention cells are done their PSUM pool joins
                # the rotation, doubling the copy/matmul overlap depth
                yps = pools[gi % len(pools)].tile([P, 512], F32, tag="ot")
                for p in range(PAIRS):
                    nc.tensor.matmul(
                        yps,
                        ot_pairs[(qb, p)][:, ts(qt_i, 128)],
                        wo_sb[:, p, ds(oh * 512, 512)],
                        start=(p == 0),
                        stop=(p == PAIRS - 1),
                    )
                ysb = y_pool.tile([P, 512], F32)
                rows = y[qb * 512 + qt_i * 128 : qb * 512 + (qt_i + 1) * 128, :]
                if last_small and gi == len(groups) - 1:
                    # very last store: halve the copy (DVE + ACT in
                    # parallel) and the store (SP + Pool in parallel) so
                    # the kernel-ending DMA latency chain is as short as
                    # possible
                    nc.vector.tensor_copy(ysb[:, 0:256], yps[:, 0:256])
                    nc.scalar.activation(
                        ysb[:, 256:512],
                        yps[:, 256:512],
                        mybir.ActivationFunctionType.Copy,
                        scale=1.0,
                    )
                    nc.sync.dma_start(rows[:, ds(oh * 512, 256)], ysb[:, 0:256])
                    nc.gpsimd.dma_start(
                        rows[:, ds(oh * 512 + 256, 256)], ysb[:, 256:512]
                    )
                    continue
                if (qt_i * 2 + oh) % act_period == act_period - 1:
                    nc.scalar.activation(
                        ysb, yps, mybir.ActivationFunctionType.Copy, scale=1.0
                    )
                else:
                    nc.vector.tensor_copy(ysb, yps)
                if (qt_i * 2 + oh) % 2 == 1:
                    nc.gpsimd.dma_start(rows[:, ds(oh * 512, 512)], ysb)
                else:
                    nc.sync.dma_start(rows[:, ds(oh * 512, 512)], ysb)

        # ---- pipeline: per s-block round, DMA + project. The in-order PE
        # needs ACT-independent filler while the exp chain of a cell-pair
        # (~4.2us ACT vs ~3.4us PE) catches up, so cell-pairs are woven
        # between projection chains: round r emits its Q chains first
        # (woven with pairs carried from round r-1), then V/K chains woven
        # with round r's own (r, kc<r) pairs (those only need Q_r), and
        # carries the (qb<=r, r) pairs into round r+1's Q phase. ----
        def pair_halves(qb, kc, p):
            """Two emission thunks for one cell-pair (shared PSUM state)."""
            state = {}
            return [
                lambda h=h: cell_pair(qb, kc, p, halves=(h,), state=state)
                for h in (0, 1)
            ]

        def to_halves(pair_thunk_specs):
            out = []
            for qb, kc, p in pair_thunk_specs:
                out.extend(pair_halves(qb, kc, p))
            return out

        def interleave(chains, pairs):
            """Emit chain thunks with pair thunks distributed by weight.

            chains is a list of (thunk, weight) where weight ~ the chain's
            PE duration; pairs are spread proportionally so a long K/Q
            chain absorbs more ACT-deficit than a short V chain.
            """
            n_p = len(pairs)
            if not chains:
                for t in pairs:
                    t()
                return
            total_w = sum(w for _, w in chains)
            emitted, cum_w = 0, 0
            for ch, w in chains:
                ch()
                cum_w += w
                want = round(cum_w * n_p / total_w)
                while emitted < want:
                    pairs[emitted]()
                    emitted += 1
            while emitted < n_p:
                pairs[emitted]()
                emitted += 1

        carry = []  # cell-pair thunks carried from the previous round
        for sb in range(SB):
            bounds = [0, 1, 4, 6, 8] if sb == 0 else [0, 2, 4, 6, 8]
            hT_tiles = []
            for i in range(4):
                lo, hi = bounds[i], bounds[i + 1]
                htt = ht_pool.tile([P, hi - lo, 512], BF16, tag=f"ht{i}", name=f"ht{i}")
                hT_tiles.append((htt, lo, hi))
            if sb == 0:
                # cold start: spread the first hidden tiles across the SP,
                # DVE and ACT queues — each engine is an independent DMA
                # pipe, so the tiles land ~in parallel instead of
                # serializing behind one queue
                engines = [nc.sync, nc.scalar, nc.sync, nc.scalar]
                for (t, lo, hi), eng in zip(hT_tiles, engines):
                    eng.dma_start(t, hiddent[:, lo:hi, ts(sb, 512)])
            else:
                for t, lo, hi in hT_tiles:
                    nc.sync.dma_start(t, hiddent[:, lo:hi, ts(sb, 512)])
            if sb == 1:
                nc.sync.dma_start(wo_sb, wot)

            def hts(dc, hT_tiles=hT_tiles):
                for t, lo, hi in hT_tiles:
                    if lo <= dc < hi:
                        return t[:, dc - lo, :]
                raise AssertionError(dc)

            def v_chain(st, sb=sb, hts=hts):
                ki_idx = sb * 4 + st
                vps = ot_ps.tile([P, 512], F32, tag="ot", name="vps")
                for dc in range(DC):
                    nc.tensor.matmul(
                        vps[:, :256],
                        hts(dc)[:, ts(st, 128)],
                        (wv_a if dc < 4 else wv_b)[:, dc % 4, :],
                        start=(dc == 0),
                        stop=(dc == DC - 1),
                    )
                for p in range(PAIRS):
                    nc.vector.tensor_copy(
                        Vp[:, ki_idx, p, 0:64], vps[:, (2 * p) * 64 : (2 * p + 1) * 64]
                    )
                    nc.vector.tensor_copy(
                        Vp[:, ki_idx, p, 65:129],
                        vps[:, (2 * p + 1) * 64 : (2 * p + 2) * 64],
                    )

            def kq_chain(w_sb, out_t, p, sb=sb, hts=hts):
                kps = ot_ps.tile([P, 512], F32, tag="ot", name="kps")
                for dc in range(DC):
                    nc.tensor.matmul(
                        kps,
                        w_sb[:, dc, ts(p, 128)],
                        hts(dc),
                        start=(dc == 0),
                        stop=(dc == DC - 1),
                    )
                nc.vector.tensor_copy(out_t[:, p, ts(sb, 512)], kps)

            q_chains = [
                (lambda p=p: kq_chain(wq_sb, QT, p), 2) for p in range(PAIRS)
            ]
            vk_chains = [(lambda st=st: v_chain(st), 1) for st in range(4)]
            vk_chains += [
                (lambda p=p: kq_chain(wk_sb, KT, p), 2) for p in range(PAIRS)
            ]
            # this round's own early pairs: q-block sb against keys kc < sb
            # (they need only Q_sb from this round's chains)
            if os.environ.get("K_HALF", "0") == "1":
                q_pairs = to_halves(
                    [(sb, kc, p) for kc in range(sb) for p in range(PAIRS)]
                )
            else:
                q_pairs = [
                    lambda kc=kc, p=p: cell_pair(sb, kc, p)
                    for kc in range(sb)
                    for p in range(PAIRS)
                ]
            # the diagonal cell (sb, sb) needs this round's V/K/Q; in the
            # last round weave it in at the end of the VK phase so the
            # final stretch has fewer filler-less pairs
            diag = [
                lambda p=p: cell_pair(sb, sb, p) for p in range(PAIRS)
            ] if (
                sb == SB - 1
                or os.environ.get("K_DIAG") == "all"
                or (sb == 0 and os.environ.get("K_DIAG0", "0") == "1")
            ) else []
            sched = os.environ.get("K_SCHED", "mixed")
            if sb == 0:
                # round 0: V/K first (smaller wv_a lands first so the PE
                # starts earlier off the cold DMA), Q at the end
                for ch, _ in vk_chains + q_chains:
                    ch()
                for t in diag:
                    t()
            elif sched == "early":
                # chains first: later rounds' cells become available as
                # early as possible at the cost of ACT idling here
                for ch, _ in q_chains + vk_chains:
                    ch()
                for t in carry + q_pairs + diag:
                    t()
            elif sb == SB - 1 and sched == "mixed":
                # last round: V3's consumers are the diagonal and (qb, 3)
                # cells, so the V chains serve as filler deep in the
                # q_pairs stretch where the ACT deficit concentrates
                k_chains = [c for c in vk_chains if c[1] == 2]
                v_chains = [c for c in vk_chains if c[1] == 1]
                n_q = min(len(carry), max(1, (len(carry) + len(q_pairs)) // 3))
                n_q -= n_q % 2  # never cut between the halves of one pair
                interleave(q_chains, carry[:n_q])
                interleave(k_chains, carry[n_q:])
                interleave(v_chains, q_pairs)
                for t in diag:
                    t()
            else:
                n_q = min(len(carry), max(1, (len(carry) + len(q_pairs)) // 4))
                n_q -= n_q % 2  # never cut between the halves of one pair
                interleave(q_chains, carry[:n_q])
                interleave(vk_chains, carry[n_q:] + q_pairs)
                for t in diag:
                    t()
            # carry (qb, sb) for qb < sb (plus the diagonal except in the
            # last round) into the next round's Q phase
            carry_specs = [
                (qb, sb, p)
                for qb in (
                    list(range(sb))
                    + (
                        [sb]
                        if sb < SB - 1
                        and os.environ.get("K_DIAG") != "all"
                        and not (sb == 0 and os.environ.get("K_DIAG0", "0") == "1")
                        else []
                    )
                )
                for p in range(PAIRS)
            ]
            if os.environ.get("K_HALF", "0") == "1":
                carry = to_halves(carry_specs)
            else:
                carry = [
                    lambda qb=qb, kc=kc, p=p: cell_pair(qb, kc, p)
                    for qb, kc, p in carry_specs
                ]

        # ---- final stretch: the (qb, 3) pairs, with normalizes emitted
        # per-pair as their data lands and output projections split in
        # halves to fill the remaining cells' ACT-wait gaps; q-block 3
        # first so its finalize unlocks the first filler ----
        g_lo = [(0, 0), (0, 1), (1, 0), (1, 1)]
        g_hi = [(2, 0), (2, 1), (3, 0), (3, 1)]
        # (3,3) already ran at the end of round 3's weave
        normalize(3, 0)
        normalize(3, 1)
        final_cells = [(0, 3), (1, 3), (2, 3)]
        post = {  # after pair (qb, kc, p) emit these filler actions
            (0, 3, 0): [lambda: normalize(0, 0), lambda: outproj(3, g_lo)],
            (0, 3, 1): [lambda: normalize(0, 1), lambda: outproj(3, g_hi)],
            (1, 3, 0): [lambda: normalize(1, 0), lambda: outproj(0, g_lo)],
            (1, 3, 1): [lambda: normalize(1, 1), lambda: outproj(0, g_hi)],
            (2, 3, 0): [lambda: normalize(2, 0)],
            (2, 3, 1): [
                lambda: normalize(2, 1, per_unit=True),
                # outproj(1) runs entirely after the last cell: it is the
                # PE filler that covers the last pair's normalize chain
                lambda: outproj(
                    1, g_lo + g_hi, act_period=2, pools=[ot_ps, cell_ps]
                ),
            ],
        }
        # `carry` out of round 3 is exactly the final_cells' pair thunks;
        # re-emit them here with the filler weave instead
        for qb, kc in final_cells:
            for p in range(PAIRS):
                cell_pair(qb, kc, p)
                for act in post.get((qb, kc, p), []):
                    act()
        outproj(2, g_lo + g_hi, act_period=2, pools=[ot_ps, cell_ps], last_small=True)
    nc.compile()
    return nc


_NC = None


def get_nc():
    global _NC
    if _NC is None:
        _NC = build_nc()
    return _NC


def shard_inputs(hidden_states, Wq, Wk, Wv, Wo):
    """Per-core input maps. Core c: batch c//4, heads 4*(c%4) .. 4*(c%4)+3."""
    import ml_dtypes

    bf16 = ml_dtypes.bfloat16
    hidden_states = np.asarray(hidden_states, np.float32)
    Wq, Wk, Wv, Wo = (np.asarray(w, np.float32) for w in (Wq, Wk, Wv, Wo))
    in_maps = []
    for c in range(N_CORES):
        b = c // 4
        f0 = (c % 4) * 4 * DIM_HEAD  # first feature row/col of this core's heads
        rows = slice(f0, f0 + UNITS * DIM_HEAD)

        def proj_layout(w):
            # W[rows].T is [D, 256]; on-chip layout is [128, DC, 256]
            return np.ascontiguousarray(
                w[rows, :].T.reshape(DC, P, 256).transpose(1, 0, 2).astype(bf16)
            )

        # Wo[:, rows].T is [256, D]; pair-packed on-chip layout [128, PAIRS, D]
        wot = np.ascontiguousarray(
            Wo[:, rows].T.reshape(PAIRS, P, D).transpose(1, 0, 2)
        )
        in_maps.append(
            {
                "hiddent": np.ascontiguousarray(
                    hidden_states[b].T.reshape(DC, P, S).transpose(1, 0, 2).astype(bf16)
                ),
                "wqt": proj_layout(Wq),
                "wkt": proj_layout(Wk),
                "wvt": proj_layout(Wv),
                "wot": wot,
            }
        )
    return in_maps


def unshard_outputs(results, bo):
    out = np.zeros((B, S, D), np.float32)
    for c, res in enumerate(results):
        out[c // 4] += res["y"]
    out += np.asarray(bo, np.float32)[None, None, :]
    return out


def kernel(hidden_states, Wq, Wk, Wv, Wo, bo, _trace=False):
    from concourse.bass_utils import run_bass_kernel_spmd

    nc = get_nc()
    in_maps = shard_inputs(hidden_states, Wq, Wk, Wv, Wo)
    res = run_bass_kernel_spmd(nc, in_maps, list(range(N_CORES)), trace=_trace)
    out = unshard_outputs(res.results, bo)
    if _trace:
        return out, res
    return out
